# revision 1
# baseline (speedup 1.0000x reference)
"""DiffAttention Trainium2 kernel.

Full inputs in, full output out. Sharding: 8 cores = (batch b in {0,1}) x
(head-pair p in {0..3}); each core handles one batch element and 2 of the 8
heads (= 4 of the 16 q/k half-heads, 2 v heads, 256 of the 1024 o columns).
Out-projection is column-split: each core produces a full (S, D) partial of
o @ Wo.T restricted to its o columns; host sums the 4 partials per batch.

All device matmuls run in fp16 (1 cycle/row on PE, fp32 PSUM accumulation).
Host pre-transposes operands so every matmul operand is loaded with natural
(contiguous) DMA:
  xT  = x[b].T                  (D, S)   rhs / lhsT for projections
  wqT = (Wq[rows].T) * hd^-0.5  (D, 256) lhsT for q^T projection (scaling folded)
  wkT = Wk[rows].T              (D, 256)
  wvT = Wv[rows].T              (D, 256) rhs for v projection
  woT = Wo[:, cols].T           (256, D) lhsT for out^T projection
Device returns outT_partial (D, S) fp16; host sums 4 partials per batch in
fp32 and transposes back.

Attention math per head h (half-heads e0=2h, e1=2h+1), per q row:
  u_i = exp(s_i) @ v   (unnormalized), sum_i = exp(s_i) @ 1  (fused: rhs=[v|1])
  o   = u0/sum0 - lam * u1/sum1
  o   = o * rsqrt(mean(o^2)+eps) * (1-lam_init);   out = o @ Wo.T
Scores are computed transposed (keys on partitions, q on free dim) so the
exp'd tiles feed the PV matmul directly as the stationary operand. rsqrt is
Newton-Raphson on the DVE (fast-inverse-sqrt seed), batched per (strip, head),
keeping the ACT engine exp-only (single activation table, no reload churn);
the (1-lam_init) factor is folded into the rsqrt argument. The main loop is
strip-major (512 q columns) so each strip's out-projection and output DMA
overlap the next strip's attention; strips run in order [0,3,1,2] (ending on
a mid-sized strip empirically minimizes the end-of-kernel drain). PSUM banks:
scores 2x2 (both half-heads
share one 1024-wide tile, exp'd by a single strided ACT op), u 2, and 2
shared by the o^T transposes and the out-projection.
"""

import math

import numpy as np

B = 2
S = 2048
D = 1024
H = 8
HD = 64  # half-head dim
LAMBDA_INIT = 0.8 - 0.6 * math.exp(-0.3 * 6)
EPS = 1e-5

N_CORES = 8
KT = D // 128      # 8 contraction tiles for projections
ST = S // 128      # 16 sequence tiles
NSTRIP = S // 512  # 4 q strips


def _build_program(lam: float, dbg: bool = False):
    import concourse.bass as bass
    import concourse.tile as tile
    from concourse import bacc, mybir
    from concourse.masks import make_identity

    f16 = mybir.dt.float16
    f32 = mybir.dt.float32
    u32 = mybir.dt.uint32
    AF = mybir.ActivationFunctionType
    OP = mybir.AluOpType

    nc = bacc.Bacc("TRN2", target_bir_lowering=False, debug=False,
                   num_devices=N_CORES)

    xT = nc.dram_tensor("xT", (D, S), f16, kind="ExternalInput").ap()
    wqT = nc.dram_tensor("wqT", (D, 256), f16, kind="ExternalInput").ap()
    wkT = nc.dram_tensor("wkT", (D, 256), f16, kind="ExternalInput").ap()
    wvT = nc.dram_tensor("wvT", (D, 256), f16, kind="ExternalInput").ap()
    woT = nc.dram_tensor("woT", (256, D), f16, kind="ExternalInput").ap()
    outT = nc.dram_tensor("outT", (D, S), f16, kind="ExternalOutput").ap()
    if dbg:
        d_qT = nc.dram_tensor("d_qT", (256, S), f16, kind="ExternalOutput").ap()
        d_kT = nc.dram_tensor("d_kT", (256, S), f16, kind="ExternalOutput").ap()
        d_v = nc.dram_tensor("d_v", (S, 258), f16, kind="ExternalOutput").ap()
        d_oT = nc.dram_tensor("d_oT", (256, S), f16, kind="ExternalOutput").ap()
        d_u = nc.dram_tensor("d_u", (S, 2, 258), f32, kind="ExternalOutput").ap()

    with tile.TileContext(nc) as tc:
        with (
            tc.tile_pool(name="const", bufs=1) as cpool,
            tc.tile_pool(name="persist", bufs=1) as pp,
        ):
            ident = cpool.tile([128, 128], f16, tag="ident")
            make_identity(nc, ident)
            # mask[p, f] = 1 if p <= f else 0 (keys on partitions, q on free)
            maskt = cpool.tile([128, 128], f16, tag="maskt")
            nc.gpsimd.memset(maskt, 1.0)
            nc.gpsimd.affine_select(
                out=maskt, in_=maskt, compare_op=OP.is_ge, fill=0.0,
                base=0, pattern=[[1, 128]], channel_multiplier=-1,
            )
            # constants for Newton-Raphson rsqrt (fast-inverse-sqrt seed)
            magic_c = cpool.tile([128, 8], u32, tag="magic_c")
            nc.gpsimd.memset(magic_c, 0x5F3759DF)
            one_u = cpool.tile([128, 8], u32, tag="one_u")
            nc.gpsimd.memset(one_u, 1)

            wo_sb = pp.tile([128, 2, D], f16, tag="wo_sb")
            qT_sb = pp.tile([128, 2, S], f16, tag="qT_sb")
            kT_sb = pp.tile([128, 2, S], f16, tag="kT_sb")
            # v with a ones column appended per head: [v_h0 | 1 | v_h1 | 1]
            v_sb = pp.tile([128, ST, 258], f16, tag="v_sb")
            nc.vector.memset(v_sb[:, :, 128:129], 1.0)
            nc.vector.memset(v_sb[:, :, 257:258], 1.0)
            oT_sb = pp.tile([128, 2, S], f16, tag="oT_sb")

            nc.gpsimd.dma_start(
                wo_sb[:, :, :],
                woT.rearrange("(kt p) n -> p kt n", p=128)[:, :, :])

            # ---------------- projections ----------------
            from contextlib import ExitStack
            pin_ctx = ExitStack()
            pin = pin_ctx.enter_context(tc.tile_pool(name="proj_in", bufs=1))
            with (
                tc.tile_pool(name="ps_qk", bufs=3, space="PSUM") as ps_qk,
                tc.tile_pool(name="ps_v", bufs=2, space="PSUM") as ps_v,
            ):
                xT_sb = pin.tile([128, KT, S], f16, tag="xT_sb")
                wq_sb = pin.tile([128, KT, 256], f16, tag="wq_sb")
                wk_sb = pin.tile([128, KT, 256], f16, tag="wk_sb")
                wv_sb = pin.tile([128, KT, 256], f16, tag="wv_sb")
                xT_r = xT.rearrange("(kt p) s -> p kt s", p=128)
                wq_r = wqT.rearrange("(kt p) m -> p kt m", p=128)
                wk_r = wkT.rearrange("(kt p) m -> p kt m", p=128)
                wv_r = wvT.rearrange("(kt p) m -> p kt m", p=128)
                nc.scalar.dma_start(wq_sb[:, :, :], wq_r[:, :, :])
                nc.scalar.dma_start(wk_sb[:, :, :], wk_r[:, :, :])
                nc.gpsimd.dma_start(wv_sb[:, :, :], wv_r[:, :, :])
                for kt2 in range(4):
                    nc.sync.dma_start(xT_sb[:, 2 * kt2:2 * kt2 + 2, :],
                                      xT_r[:, 2 * kt2:2 * kt2 + 2, :])

                def proj_qk(w_sb, dst_sb, mt, evac_engine):
                    for half in range(2):
                        ps = ps_qk.tile([128, 1024], f32, tag="qk")
                        for kt in range(KT):
                            for ns in range(2):
                                nc.tensor.matmul(
                                    ps[:, ns * 512:(ns + 1) * 512],
                                    lhsT=w_sb[:, kt, mt * 128:(mt + 1) * 128],
                                    rhs=xT_sb[:, kt,
                                              half * 1024 + ns * 512:
                                              half * 1024 + (ns + 1) * 512],
                                    start=(kt == 0), stop=(kt == KT - 1),
                                )
                        dst = dst_sb[:, mt, half * 1024:(half + 1) * 1024]
                        if evac_engine == "act":
                            nc.scalar.copy(dst, ps[:])
                        else:
                            nc.vector.tensor_copy(dst, ps[:])

                proj_qk(wq_sb, qT_sb, 0, "vector")
                proj_qk(wk_sb, kT_sb, 0, "vector")
                proj_qk(wq_sb, qT_sb, 1, "vector")
                proj_qk(wk_sb, kT_sb, 1, "vector")

                for st in range(ST):
                    ps = ps_v.tile([128, 256], f32, tag="v")
                    for kt in range(KT):
                        nc.tensor.matmul(
                            ps[:],
                            lhsT=xT_sb[:, kt, st * 128:(st + 1) * 128],
                            rhs=wv_sb[:, kt, :],
                            start=(kt == 0), stop=(kt == KT - 1),
                        )
                    nc.vector.tensor_copy(v_sb[:, st, 0:128], ps[:, 0:128])
                    nc.vector.tensor_copy(v_sb[:, st, 129:257], ps[:, 128:256])

            if dbg:
                d_qT_r = d_qT.rearrange("(mt p) s -> p mt s", p=128)
                d_kT_r = d_kT.rearrange("(mt p) s -> p mt s", p=128)
                d_v_r = d_v.rearrange("(st p) c -> p st c", p=128)
                for mt in range(2):
                    nc.sync.dma_start(d_qT_r[:, mt, :], qT_sb[:, mt, :])
                    nc.sync.dma_start(d_kT_r[:, mt, :], kT_sb[:, mt, :])
                for st in range(ST):
                    nc.sync.dma_start(d_v_r[:, st, :], v_sb[:, st, :])

            pin_ctx.close()

            # ---------------- attention + per-strip out projection ----------
            with (
                tc.tile_pool(name="e0p", bufs=20) as e0pool,
                tc.tile_pool(name="e1p", bufs=20) as e1pool,
                tc.tile_pool(name="ps_s", bufs=2, space="PSUM") as ps_s,
                tc.tile_pool(name="ps_u", bufs=2, space="PSUM") as ps_u,
                tc.tile_pool(name="ps_o", bufs=2, space="PSUM") as ps_o,
                tc.tile_pool(name="nrm", bufs=8) as nrm,
                tc.tile_pool(name="nrm_big", bufs=3) as nrm_big,
                tc.tile_pool(name="osb", bufs=3) as osb,
                tc.tile_pool(name="out_sb", bufs=2) as out_pool,
            ):
                epools = {0: e0pool, 1: e1pool}
                outT_r = outT.rearrange("(mt p) s -> p mt s", p=128)

                def pv_qtile(h, s, i, e0_tiles, e1_tiles, oq_s, ss_s):
                    qt = 4 * s + i
                    up = ps_u.tile([128, 258], f32, tag="u")
                    for kt in range(qt + 1):
                        c = i * 128
                        vh = v_sb[:, kt, 129 * h:129 * h + 129]
                        nc.tensor.matmul(
                            up[:, 0:129],
                            lhsT=e0_tiles[kt][:, c:c + 128],
                            rhs=vh,
                            start=(kt == 0), stop=(kt == qt),
                        )
                        nc.tensor.matmul(
                            up[:, 129:258],
                            lhsT=e1_tiles[kt][:, 512 + c:512 + c + 128],
                            rhs=vh,
                            start=False, stop=(kt == qt),
                            skip_group_check=True,
                        )
                    # normalized diff: oq = u0/s0 - lam*u1/s1 (per-partition)
                    inv0 = nrm.tile([128, 1], f32, tag="inv0")
                    nc.vector.reciprocal(inv0, up[:, 128:129])
                    inv1 = nrm.tile([128, 1], f32, tag="inv1")
                    nc.vector.reciprocal(inv1, up[:, 257:258])
                    t1 = nrm.tile([128, 128], f32, tag="t1")
                    nc.vector.tensor_scalar(t1, up[:, 129:257], inv1, lam,
                                            OP.mult, OP.mult)
                    oq = oq_s[:, i, :]
                    nc.vector.scalar_tensor_tensor(
                        oq, up[:, 0:128], inv0, t1, OP.mult, OP.subtract)
                    sq = nrm.tile([128, 128], f32, tag="sq")
                    nc.vector.scalar_tensor_tensor(
                        sq, oq, 1.0, oq, OP.bypass, OP.mult,
                        accum_out=ss_s[:, i:i + 1])
                    if dbg:
                        ub = nrm.tile([128, 258], f32, tag="ub")
                        nc.vector.tensor_copy(ub, up[:])
                        nc.sync.dma_start(
                            d_u.rearrange("(qt p) h c -> p qt h c",
                                          p=128)[:, qt, h, :], ub[:])

                def norm_tail(h, s, oq_s, ss_s):
                    """Newton rsqrt over the strip's 4 q-tiles, then scale,
                    transpose and evacuate o^T."""
                    ms = nrm.tile([128, 4], f32, tag="ms")
                    il2 = 1.0 / ((1.0 - LAMBDA_INIT) ** 2)
                    nc.vector.tensor_scalar(ms, ss_s, il2 / 128.0, EPS * il2,
                                            OP.mult, OP.add)
                    y0 = nrm.tile([128, 4], u32, tag="y0")
                    nc.vector.tensor_tensor(y0, ms.bitcast(u32),
                                            one_u[:, 0:4],
                                            OP.logical_shift_right)
                    nc.vector.tensor_tensor(y0, magic_c[:, 0:4], y0,
                                            OP.subtract)
                    yf = y0.bitcast(f32)
                    t2 = nrm.tile([128, 4], f32, tag="t2")
                    r_all = nrm.tile([128, 4], f32, tag="r_all")
                    nc.vector.tensor_mul(t2, yf, yf)
                    nc.vector.tensor_mul(t2, t2, ms)
                    nc.vector.tensor_scalar(t2, t2, -0.5, 1.5, OP.mult, OP.add)
                    nc.vector.tensor_mul(r_all, yf, t2)
                    nc.vector.tensor_mul(t2, r_all, r_all)
                    nc.vector.tensor_mul(t2, t2, ms)
                    nc.vector.tensor_scalar(t2, t2, -0.5, 1.5, OP.mult, OP.add)
                    nc.vector.tensor_mul(r_all, r_all, t2)
                    for i in range(4):
                        qt = 4 * s + i
                        on = osb.tile([128, 128], f16, tag="on")
                        nc.vector.tensor_scalar(on, oq_s[:, i, :],
                                                r_all[:, i:i + 1], None,
                                                OP.mult)
                        pt = ps_o.tile([128, 128], f16, tag="o")
                        nc.tensor.transpose(pt, on, ident)
                        nc.vector.tensor_copy(
                            oT_sb[:, h, qt * 128:(qt + 1) * 128], pt[:])

                def emit_outproj(s):
                    ot = out_pool.tile([128, 8, 512], f16, tag="ot")
                    for mt in range(8):
                        ps = ps_o.tile([128, 512], f32, tag="o")
                        for kt in range(2):
                            nc.tensor.matmul(
                                ps[:],
                                lhsT=wo_sb[:, kt, mt * 128:(mt + 1) * 128],
                                rhs=oT_sb[:, kt, s * 512:(s + 1) * 512],
                                start=(kt == 0), stop=(kt == 1),
                            )
                        nc.vector.tensor_copy(ot[:, mt, :], ps[:])
                    nc.sync.dma_start(outT_r[:, :, s * 512:(s + 1) * 512], ot[:])

                for si, s in enumerate([0, 3, 1, 2]):
                    for h in range(2):
                        e0_tiles = {}
                        e1_tiles = {}
                        oq_s = nrm_big.tile([128, 4, 128], f32, tag="oq_s")
                        ss_s = nrm_big.tile([128, 4], f32, tag="ss_s")
                        for kt in range(4 * (s + 1)):
                            col0 = max(0, (kt - 4 * s) * 128)
                            pa = ps_s.tile([128, 1024], f32, tag="sc")
                            nc.tensor.matmul(
                                pa[:, col0:512],
                                lhsT=kT_sb[0:64, h, kt * 128:(kt + 1) * 128],
                                rhs=qT_sb[0:64, h, s * 512 + col0:(s + 1) * 512],
                                start=True, stop=True, tile_position=(0, 0),
                            )
                            nc.tensor.matmul(
                                pa[:, 512 + col0:1024],
                                lhsT=kT_sb[64:128, h, kt * 128:(kt + 1) * 128],
                                rhs=qT_sb[64:128, h, s * 512 + col0:(s + 1) * 512],
                                start=True, stop=True, tile_position=(64, 0),
                                skip_group_check=True,
                            )
                            ee = epools[h].tile([128, 1024], f16, tag="e")
                            # exp both half-heads in one ACT op (strided AP
                            # skips the invalid leading columns of each half)
                            w_ = 512 - col0
                            nc.scalar.activation(
                                ee.rearrange("p (b c) -> p b c", b=2)[:, :, col0:512],
                                pa.rearrange("p (b c) -> p b c", b=2)[:, :, col0:512],
                                AF.Exp)
                            if kt >= 4 * s:
                                c = col0
                                nc.gpsimd.tensor_mul(ee[:, c:c + 128],
                                                     ee[:, c:c + 128], maskt)
                                nc.gpsimd.tensor_mul(ee[:, 512 + c:512 + c + 128],
                                                     ee[:, 512 + c:512 + c + 128],
                                                     maskt)
                            e0_tiles[kt] = ee
                            e1_tiles[kt] = ee
                            if kt >= 4 * s:
                                pv_qtile(h, s, kt - 4 * s, e0_tiles, e1_tiles,
                                         oq_s, ss_s)
                        norm_tail(h, s, oq_s, ss_s)

                    emit_outproj(s)

            if dbg:
                d_oT_r = d_oT.rearrange("(mt p) s -> p mt s", p=128)
                for mt in range(2):
                    nc.sync.dma_start(d_oT_r[:, mt, :], oT_sb[:, mt, :])

    nc.compile()
    return nc


def _prep_inputs(x, Wq, Wk, Wv, Wo):
    """Build the 8 per-core input maps (host-side shard + transpose)."""
    f16 = np.float16
    xT = [np.ascontiguousarray(x[b].T).astype(f16) for b in range(B)]
    scale = HD ** -0.5
    in_maps = []
    for d in range(N_CORES):
        b, p = divmod(d, 4)
        r0 = 256 * p
        in_maps.append({
            "xT": xT[b],
            "wqT": np.ascontiguousarray(Wq[r0:r0 + 256, :].T * scale).astype(f16),
            "wkT": np.ascontiguousarray(Wk[r0:r0 + 256, :].T).astype(f16),
            "wvT": np.ascontiguousarray(Wv[r0:r0 + 256, :].T).astype(f16),
            "woT": np.ascontiguousarray(Wo[:, r0:r0 + 256].T).astype(f16),
        })
    return in_maps


_CACHED = {}


def _get_program(lam: float):
    # the program depends on inputs only through lam
    key = round(float(lam), 9)
    if key not in _CACHED:
        _CACHED[key] = _build_program(float(lam))
    return _CACHED[key]


def kernel(x, Wq, Wk, Wv, Wo, lq1, lk1, lq2, lk2):
    from concourse.bass_utils import run_bass_kernel_spmd

    x = np.asarray(x, dtype=np.float32)
    Wq = np.asarray(Wq, dtype=np.float32)
    Wk = np.asarray(Wk, dtype=np.float32)
    Wv = np.asarray(Wv, dtype=np.float32)
    Wo = np.asarray(Wo, dtype=np.float32)
    lq1 = np.asarray(lq1, dtype=np.float32)
    lk1 = np.asarray(lk1, dtype=np.float32)
    lq2 = np.asarray(lq2, dtype=np.float32)
    lk2 = np.asarray(lk2, dtype=np.float32)

    lam1 = np.exp(np.sum(lq1 * lk1, dtype=np.float32))
    lam2 = np.exp(np.sum(lq2 * lk2, dtype=np.float32))
    lam = float(lam1 - lam2 + LAMBDA_INIT)

    nc = _get_program(lam)
    in_maps = _prep_inputs(x, Wq, Wk, Wv, Wo)
    res = run_bass_kernel_spmd(nc, in_maps, core_ids=list(range(N_CORES)))

    out = np.empty((B, S, D), dtype=np.float32)
    for b in range(B):
        acc = res.results[4 * b]["outT"].astype(np.float32)
        for p in range(1, 4):
            acc += res.results[4 * b + p]["outT"].astype(np.float32)
        out[b] = acc.T
    return out



# revision 21
# speedup vs baseline: 1.1377x; 1.1377x over previous
"""DiffAttention Trainium2 kernel (v2).

Full inputs in, full output out. Sharding: 8 cores = (batch b in {0,1}) x
(head-pair p in {0..3}); each core handles one batch element and 2 of the 8
heads (= 4 of the 16 q/k half-heads, 2 v heads, 256 of the 1024 o columns).
Out-projection is column-split: each core produces a full (S, D) partial of
o @ Wo.T restricted to its o columns; host sums the 4 partials per batch.

v2 changes over the baseline:
- q/k/v projections run in fp8e4m3 with DoubleRow perf mode (0.5 cyc/row,
  256-deep contraction). Accuracy is restored with a 3-chain residual
  decomposition: with x~=(x8+r8)/b and W~=(W8+rW8)/a (operands pre-scaled
  into fp8's sweet range host-side), x@W = [x8@W8 + (x8@rW8 + r8@W8)]/(ab),
  dropping the O(eps^2) r8@rW8 term. The main chain is 4 DoubleRow matmuls
  pairing kt tiles of x8/W8; the correction is 8 DoubleRow matmuls pairing
  (x8,r8) against (rW8,W8) per kt — together 12 DR matmuls (6 eff. rows)
  versus 8 fp16 matmuls, a 25% PE saving at ~2e-3 overall rel err.
- Projections are emitted per 512-column S-chunk and software-pipelined into
  the attention strip loop (strip s consumes chunk s; chunk s+1's projection
  groups are interleaved between strip s's kt iterations). This overlaps the
  exp-heavy ACT phase with projection PE work and fills PE dependency
  bubbles with independent projection matmuls.
- Evacuation copies use nc.any so the Tile scheduler balances DVE/ACT.
- Input DMAs avoid the ACT queue (exp is the second-busiest engine);
  weight/x DMAs are split fine-grained so the first projection starts ~3us
  in; output DMA is split in half per strip to shorten the kernel tail.

Attention math per head h (half-heads e0=2h, e1=2h+1), per q row:
  u_i = exp(s_i) @ v   (unnormalized), sum_i = exp(s_i) @ 1  (fused: rhs=[v|1])
  o   = u0/sum0 - lam * u1/sum1
  o   = o * rsqrt(mean(o^2)+eps) * (1-lam_init);   out = o @ Wo.T
Scores are computed transposed (keys on partitions, q on free dim) so the
exp'd tiles feed the PV matmul directly as the stationary operand. rsqrt is
Newton-Raphson on the DVE (fast-inverse-sqrt seed), batched per (strip, head),
keeping the ACT engine exp-only. PSUM banks: scores 2x2, u 2x1, and a shared
2x1 pool for projection chunks, o^T transposes and the out-projection.
"""

import math

import numpy as np

B = 2
S = 2048
D = 1024
H = 8
HD = 64  # half-head dim
LAMBDA_INIT = 0.8 - 0.6 * math.exp(-0.3 * 6)
EPS = 1e-5

N_CORES = 8
KT = D // 128       # 8 contraction tiles for projections
ST = S // 128       # 16 sequence tiles
NSTRIP = S // 512   # 4 q strips

BX = 16.0     # x pre-quantization scale
AQ = 1024.0   # Wq.T * hd^-0.5 scale
AK = 128.0    # Wk.T scale
AV = 128.0    # Wv.T scale


def _build_program(lam: float):
    import concourse.bass as bass
    import concourse.tile as tile
    from concourse import bacc, mybir
    from concourse.masks import make_identity

    f16 = mybir.dt.float16
    f32 = mybir.dt.float32
    f8 = mybir.dt.float8e4
    u32 = mybir.dt.uint32
    AF = mybir.ActivationFunctionType
    OP = mybir.AluOpType
    DR = mybir.MatmulPerfMode.DoubleRow

    nc = bacc.Bacc("TRN2", target_bir_lowering=False, debug=False,
                   num_devices=N_CORES)

    # packed fp8 inputs: var index 0 = x8 (scaled main), 1 = r8 (residual)
    # weights:           var index 0 = rW8 (residual), 1 = W8 (scaled main)
    xpack = nc.dram_tensor("xpack", (2, D, S), f8, kind="ExternalInput").ap()
    wqp = nc.dram_tensor("wqp", (2, D, 256), f8, kind="ExternalInput").ap()
    wkp = nc.dram_tensor("wkp", (2, D, 256), f8, kind="ExternalInput").ap()
    wvp = nc.dram_tensor("wvp", (2, D, 256), f8, kind="ExternalInput").ap()
    woT = nc.dram_tensor("woT", (256, D), f16, kind="ExternalInput").ap()
    outT = nc.dram_tensor("outT", (D, S), f16, kind="ExternalOutput").ap()

    with tile.TileContext(nc) as tc:
        with (
            tc.tile_pool(name="const", bufs=1) as cpool,
            tc.tile_pool(name="persist", bufs=1) as pp,
        ):
            ident = cpool.tile([128, 128], f16, tag="ident")
            make_identity(nc, ident)
            # mask[p, f] = 1 if p <= f else 0 (keys on partitions, q on free)
            maskt = cpool.tile([128, 128], f16, tag="maskt")
            nc.gpsimd.memset(maskt, 1.0)
            nc.gpsimd.affine_select(
                out=maskt, in_=maskt, compare_op=OP.is_ge, fill=0.0,
                base=0, pattern=[[1, 128]], channel_multiplier=-1,
            )

            # constants for Newton-Raphson rsqrt (fast-inverse-sqrt seed)
            magic_c = cpool.tile([128, 8], u32, tag="magic_c")
            nc.gpsimd.memset(magic_c, 0x5F3759DF)
            one_u = cpool.tile([128, 8], u32, tag="one_u")
            nc.gpsimd.memset(one_u, 1)

            wo_sb = pp.tile([128, 2, D], f16, tag="wo_sb")
            qT_sb = pp.tile([128, 2, S], f16, tag="qT_sb")
            kT_sb = pp.tile([128, 2, S], f16, tag="kT_sb")
            # v with a ones column appended per head: [v_h0 | 1 | v_h1 | 1]
            v_sb = pp.tile([128, ST, 258], f16, tag="v_sb")
            nc.vector.memset(v_sb[:, :, 128:129], 1.0)
            nc.vector.memset(v_sb[:, :, 257:258], 1.0)
            oT_sb = pp.tile([128, 2, S], f16, tag="oT_sb")

            # fp8 packed inputs stay resident for the whole kernel
            xp_sb = pp.tile([128, 2, KT, S], f8, tag="xp_sb")
            wq_sb = pp.tile([128, 2, KT, 256], f8, tag="wq_sb")
            wk_sb = pp.tile([128, 2, KT, 256], f8, tag="wk_sb")
            wv_sb = pp.tile([128, 2, KT, 256], f8, tag="wv_sb")

            xp_r = xpack.rearrange("v (kt p) s -> p v kt s", p=128)
            wq_r = wqp.rearrange("v (kt p) m -> p v kt m", p=128)
            wk_r = wkp.rearrange("v (kt p) m -> p v kt m", p=128)
            wv_r = wvp.rearrange("v (kt p) m -> p v kt m", p=128)

            # weight DMAs on Pool queue (ACT must stay exp-only), x chunks on
            # SP, main (v=0) halves before residual (v=1) so the first
            # projection matmuls can start early.
            nc.gpsimd.dma_start(wq_sb[:, 1, 0:2, :], wq_r[:, 1, 0:2, :])
            nc.gpsimd.dma_start(wq_sb[:, 1, 2:8, :], wq_r[:, 1, 2:8, :])
            nc.gpsimd.dma_start(wk_sb[:, 1, :, :], wk_r[:, 1, :, :])
            nc.gpsimd.dma_start(wq_sb[:, 0, :, :], wq_r[:, 0, :, :])
            nc.gpsimd.dma_start(wk_sb[:, 0, :, :], wk_r[:, 0, :, :])
            nc.sync.dma_start(xp_sb[:, 0, 0:2, 0:512], xp_r[:, 0, 0:2, 0:512])
            nc.sync.dma_start(xp_sb[:, 0, 2:8, 0:512], xp_r[:, 0, 2:8, 0:512])
            nc.sync.dma_start(xp_sb[:, 1, :, 0:512], xp_r[:, 1, :, 0:512])
            for v in (1, 0):
                nc.gpsimd.dma_start(wv_sb[:, v, :, :], wv_r[:, v, :, :])
            for c in range(1, 4):
                sl = slice(c * 512, (c + 1) * 512)
                nc.sync.dma_start(xp_sb[:, 0, :, sl], xp_r[:, 0, :, sl])
                nc.sync.dma_start(xp_sb[:, 1, :, sl], xp_r[:, 1, :, sl])
            nc.gpsimd.dma_start(
                wo_sb[:, :, :],
                woT.rearrange("(kt p) n -> p kt n", p=128)[:, :, :])

            with (
                tc.tile_pool(name="e0p", bufs=17) as e0pool,
                tc.tile_pool(name="e1p", bufs=17) as e1pool,
                tc.tile_pool(name="ps_s", bufs=2, space="PSUM") as ps_s,
                tc.tile_pool(name="ps_u", bufs=2, space="PSUM") as ps_u,
                tc.tile_pool(name="po", bufs=2, space="PSUM") as po,
                tc.tile_pool(name="nrm", bufs=8) as nrm,
                tc.tile_pool(name="nrm_big", bufs=3) as nrm_big,
                tc.tile_pool(name="osb", bufs=3) as osb,
                tc.tile_pool(name="out_sb", bufs=2) as out_pool,
            ):
                epools = {0: e0pool, 1: e1pool}
                outT_r = outT.rearrange("(mt p) s -> p mt s", p=128)

                # ---------------- fp8 projection groups ----------------
                def emit_proj_qk(w_sb, dst_sb, mt, c, inv_ab):
                    ps = po.tile([128, 512], f32, tag="po")
                    msl = slice(mt * 128, (mt + 1) * 128)
                    csl = slice(c * 512, (c + 1) * 512)
                    for j in range(4):
                        nc.tensor.matmul(
                            ps[:],
                            lhsT=w_sb[:, 1, 2 * j:2 * j + 2, msl],
                            rhs=xp_sb[:, 0, 2 * j:2 * j + 2, csl],
                            start=(j == 0), stop=False, perf_mode=DR,
                        )
                    for kt in range(KT):
                        nc.tensor.matmul(
                            ps[:],
                            lhsT=w_sb[:, :, kt, msl],
                            rhs=xp_sb[:, :, kt, csl],
                            start=False, stop=(kt == KT - 1), perf_mode=DR,
                        )
                    nc.vector.tensor_scalar(dst_sb[:, mt, csl], ps[:], inv_ab,
                                            None, OP.mult)

                def emit_proj_v(st, inv_ab):
                    ps = po.tile([128, 512], f32, tag="po")
                    ssl = slice(st * 128, (st + 1) * 128)
                    for j in range(4):
                        nc.tensor.matmul(
                            ps[:, 0:256],
                            lhsT=xp_sb[:, 0, 2 * j:2 * j + 2, ssl],
                            rhs=wv_sb[:, 1, 2 * j:2 * j + 2, :],
                            start=(j == 0), stop=False, perf_mode=DR,
                        )
                    for kt in range(KT):
                        nc.tensor.matmul(
                            ps[:, 0:256],
                            lhsT=xp_sb[:, :, kt, ssl],
                            rhs=wv_sb[:, :, kt, :],
                            start=False, stop=(kt == KT - 1), perf_mode=DR,
                        )
                    nc.vector.tensor_scalar(v_sb[:, st, 0:128], ps[:, 0:128],
                                            inv_ab, None, OP.mult)
                    nc.vector.tensor_scalar(v_sb[:, st, 129:257],
                                            ps[:, 128:256], inv_ab, None,
                                            OP.mult)

                def proj_chunk_groups(c):
                    gs = []
                    for mt in range(2):
                        gs.append(lambda mt=mt: emit_proj_qk(
                            wq_sb, qT_sb, mt, c, 1.0 / (AQ * BX)))
                        gs.append(lambda mt=mt: emit_proj_qk(
                            wk_sb, kT_sb, mt, c, 1.0 / (AK * BX)))
                    for st in range(4 * c, 4 * c + 4):
                        gs.append(lambda st=st: emit_proj_v(st, 1.0 / (AV * BX)))
                    return gs

                # ---------------- attention ----------------
                def pv_qtile(h, s, i, e_tiles, oq_s, ss_s):
                    qt = 4 * s + i
                    up = ps_u.tile([128, 258], f32, tag="u")
                    for kt in range(qt + 1):
                        c = i * 128
                        vh = v_sb[:, kt, 129 * h:129 * h + 129]
                        nc.tensor.matmul(
                            up[:, 0:129],
                            lhsT=e_tiles[kt][:, c:c + 128],
                            rhs=vh,
                            start=(kt == 0), stop=(kt == qt),
                        )
                        nc.tensor.matmul(
                            up[:, 129:258],
                            lhsT=e_tiles[kt][:, 512 + c:512 + c + 128],
                            rhs=vh,
                            start=False, stop=(kt == qt),
                            skip_group_check=True,
                        )
                    # normalized diff: oq = u0/s0 - lam*u1/s1 (per-partition)
                    inv0 = nrm.tile([128, 1], f32, tag="inv0")
                    nc.vector.reciprocal(inv0, up[:, 128:129])
                    inv1 = nrm.tile([128, 1], f32, tag="inv1")
                    nc.vector.reciprocal(inv1, up[:, 257:258])
                    t1 = nrm.tile([128, 128], f32, tag="t1")
                    nc.vector.tensor_scalar(t1, up[:, 129:257], inv1, lam,
                                            OP.mult, OP.mult)
                    oq = oq_s[:, i, :]
                    nc.vector.scalar_tensor_tensor(
                        oq, up[:, 0:128], inv0, t1, OP.mult, OP.subtract)
                    sq = nrm.tile([128, 128], f32, tag="sq")
                    nc.vector.scalar_tensor_tensor(
                        sq, oq, 1.0, oq, OP.bypass, OP.mult,
                        accum_out=ss_s[:, i:i + 1])

                def norm_tail(h, s, oq_s, ss_s, tpool=None, ttag="po"):
                    """Newton rsqrt over the strip's 4 q-tiles, then scale,
                    transpose and evacuate o^T."""
                    if tpool is None:
                        tpool = po
                    ms = nrm.tile([128, 4], f32, tag="ms")
                    il2 = 1.0 / ((1.0 - LAMBDA_INIT) ** 2)
                    nc.vector.tensor_scalar(ms, ss_s, il2 / 128.0, EPS * il2,
                                            OP.mult, OP.add)
                    y0 = nrm.tile([128, 4], u32, tag="y0")
                    nc.vector.tensor_tensor(y0, ms.bitcast(u32),
                                            one_u[:, 0:4],
                                            OP.logical_shift_right)
                    nc.vector.tensor_tensor(y0, magic_c[:, 0:4], y0,
                                            OP.subtract)
                    yf = y0.bitcast(f32)
                    t2 = nrm.tile([128, 4], f32, tag="t2")
                    r_all = nrm.tile([128, 4], f32, tag="r_all")
                    nc.vector.tensor_mul(t2, yf, yf)
                    nc.vector.tensor_mul(t2, t2, ms)
                    nc.vector.tensor_scalar(t2, t2, -0.5, 1.5, OP.mult, OP.add)
                    nc.vector.tensor_mul(r_all, yf, t2)
                    nc.vector.tensor_mul(t2, r_all, r_all)
                    nc.vector.tensor_mul(t2, t2, ms)
                    nc.vector.tensor_scalar(t2, t2, -0.5, 1.5, OP.mult, OP.add)
                    nc.vector.tensor_mul(r_all, r_all, t2)
                    for i in range(4):
                        qt = 4 * s + i
                        on = osb.tile([128, 128], f16, tag="on")
                        nc.vector.tensor_scalar(on, oq_s[:, i, :],
                                                r_all[:, i:i + 1], None,
                                                OP.mult)
                        pt = tpool.tile([128, 512], f16, tag=ttag)
                        nc.tensor.transpose(pt[:, 0:128], on, ident)
                        nc.vector.tensor_copy(
                            oT_sb[:, h, qt * 128:(qt + 1) * 128], pt[:, 0:128])

                def outproj_groups(s):
                    """8 filler groups (one per mt) computing strip s's
                    out-projection; interleaved into strip s+1."""
                    state = {}

                    def grp(mt):
                        if mt == 0:
                            ot = out_pool.tile([128, 8, 512], f16, tag="ot")
                            state["ot"] = ot
                        ot = state["ot"]
                        ps = po.tile([128, 512], f32, tag="po")
                        for kt in range(2):
                            nc.tensor.matmul(
                                ps[:],
                                lhsT=wo_sb[:, kt, mt * 128:(mt + 1) * 128],
                                rhs=oT_sb[:, kt, s * 512:(s + 1) * 512],
                                start=(kt == 0), stop=(kt == 1),
                            )
                        nc.vector.tensor_copy(ot[:, mt, :], ps[:])
                        if mt == 3:
                            nc.sync.dma_start(
                                outT_r[:, 0:4, s * 512:(s + 1) * 512],
                                ot[:, 0:4, :])
                        elif mt == 7:
                            nc.sync.dma_start(
                                outT_r[:, 4:8, s * 512:(s + 1) * 512],
                                ot[:, 4:8, :])

                    return [lambda mt=mt: grp(mt) for mt in range(8)]

                def outproj_last(s):
                    """Final strip: evacs alternate DVE/ACT; DMA per 2 mt."""
                    ot = out_pool.tile([128, 8, 512], f16, tag="ot")
                    for mt in range(8):
                        ps = po.tile([128, 512], f32, tag="po")
                        for kt in range(2):
                            nc.tensor.matmul(
                                ps[:],
                                lhsT=wo_sb[:, kt, mt * 128:(mt + 1) * 128],
                                rhs=oT_sb[:, kt, s * 512:(s + 1) * 512],
                                start=(kt == 0), stop=(kt == 1),
                            )
                        if mt % 2 == 1:
                            nc.scalar.copy(ot[:, mt, :], ps[:])
                            nc.sync.dma_start(
                                outT_r[:, mt - 1:mt + 1,
                                       s * 512:(s + 1) * 512],
                                ot[:, mt - 1:mt + 1, :])
                        else:
                            nc.vector.tensor_copy(ot[:, mt, :], ps[:])

                # chunk 0 projections up front
                for g in proj_chunk_groups(0):
                    g()

                for s in range(NSTRIP):
                    pending = []
                    if s > 0:
                        pending += outproj_groups(s - 1)
                    if s < NSTRIP - 1:
                        pending += proj_chunk_groups(s + 1)
                    niter = 2 * 4 * (s + 1)
                    total0 = len(pending)
                    emitted = 0
                    it = 0
                    e_tiles = {0: {}, 1: {}}
                    oq_ss = {}
                    for h in range(2):
                        oq_s = nrm_big.tile([128, 4, 128], f32, tag="oq_s")
                        ss_s = nrm_big.tile([128, 4], f32, tag="ss_s")
                        oq_ss[h] = (oq_s, ss_s)
                    # heads interleaved at kt granularity: h0's PV bursts
                    # overlap h1's score/exp stretch and vice versa
                    for kt in range(4 * (s + 1)):
                        col0 = max(0, (kt - 4 * s) * 128)
                        for h in range(2):
                            pa = ps_s.tile([128, 1024], f32, tag="sc")
                            nc.tensor.matmul(
                                pa[:, col0:512],
                                lhsT=kT_sb[0:64, h, kt * 128:(kt + 1) * 128],
                                rhs=qT_sb[0:64, h, s * 512 + col0:(s + 1) * 512],
                                start=True, stop=True, tile_position=(0, 0),
                            )
                            nc.tensor.matmul(
                                pa[:, 512 + col0:1024],
                                lhsT=kT_sb[64:128, h, kt * 128:(kt + 1) * 128],
                                rhs=qT_sb[64:128, h, s * 512 + col0:(s + 1) * 512],
                                start=True, stop=True, tile_position=(64, 0),
                                skip_group_check=True,
                            )
                            ee = epools[h].tile([128, 1024], f16, tag="e")
                            # exp both half-heads in one ACT op (strided AP
                            # skips the invalid leading columns of each half)
                            nc.scalar.activation(
                                ee.rearrange("p (b c) -> p b c", b=2)[:, :, col0:512],
                                pa.rearrange("p (b c) -> p b c", b=2)[:, :, col0:512],
                                AF.Exp)
                            if kt >= 4 * s:
                                c = col0
                                nc.gpsimd.tensor_mul(ee[:, c:c + 128],
                                                     ee[:, c:c + 128], maskt)
                                nc.gpsimd.tensor_mul(ee[:, 512 + c:512 + c + 128],
                                                     ee[:, 512 + c:512 + c + 128],
                                                     maskt)
                            e_tiles[h][kt] = ee
                            if kt >= 4 * s:
                                i = kt - 4 * s
                                oq_s, ss_s = oq_ss[h]
                                pv_qtile(h, s, i, e_tiles[h], oq_s, ss_s)
                                if i == 3:
                                    if s == NSTRIP - 1 and h == 1:
                                        norm_tail(h, s, oq_s, ss_s,
                                                  tpool=ps_s, ttag="sc")
                                    else:
                                        norm_tail(h, s, oq_s, ss_s)
                            # interleave deferred work
                            it += 1
                            target = -(-total0 * it // niter)  # ceil
                            while pending and emitted < target:
                                pending.pop(0)()
                                emitted += 1

                outproj_last(NSTRIP - 1)

    nc.compile()
    return nc


def _prep_inputs(x, Wq, Wk, Wv, Wo):
    """Build the 8 per-core input maps (host-side shard + fp8 packing)."""
    import ml_dtypes
    F8 = ml_dtypes.float8_e4m3
    f16 = np.float16

    def q8(a):
        return a.astype(F8)

    scale = HD ** -0.5
    # x packs are per batch: [D, 2, S] with var 0 = x8, var 1 = r8
    xpacks = []
    for b in range(B):
        xT = np.ascontiguousarray(x[b].T) * BX     # (D, S) fp32
        x8 = q8(xT)
        r8 = q8(xT - x8.astype(np.float32))
        xp = np.empty((2, D, S), dtype=F8)
        xp[0] = x8
        xp[1] = r8
        xpacks.append(xp)

    def wpack(W, a, pre=1.0):
        # W slice already (rows 256, D); computes pack of (W.T * pre) * a
        WT = np.ascontiguousarray(W.T) * (pre * a)  # (D, 256)
        W8 = q8(WT)
        rW8 = q8(WT - W8.astype(np.float32))
        wp = np.empty((2, D, 256), dtype=F8)
        wp[0] = rW8
        wp[1] = W8
        return wp

    in_maps = []
    for d in range(N_CORES):
        b, p = divmod(d, 4)
        r0 = 256 * p
        in_maps.append({
            "xpack": xpacks[b],
            "wqp": wpack(Wq[r0:r0 + 256, :], AQ, pre=scale),
            "wkp": wpack(Wk[r0:r0 + 256, :], AK),
            "wvp": wpack(Wv[r0:r0 + 256, :], AV),
            "woT": np.ascontiguousarray(Wo[:, r0:r0 + 256].T).astype(f16),
        })
    return in_maps


_CACHED = {}


def _get_program(lam: float):
    # the program depends on inputs only through lam
    key = round(float(lam), 9)
    if key not in _CACHED:
        _CACHED[key] = _build_program(float(lam))
    return _CACHED[key]


def kernel(x, Wq, Wk, Wv, Wo, lq1, lk1, lq2, lk2):
    from concourse.bass_utils import run_bass_kernel_spmd

    x = np.asarray(x, dtype=np.float32)
    Wq = np.asarray(Wq, dtype=np.float32)
    Wk = np.asarray(Wk, dtype=np.float32)
    Wv = np.asarray(Wv, dtype=np.float32)
    Wo = np.asarray(Wo, dtype=np.float32)
    lq1 = np.asarray(lq1, dtype=np.float32)
    lk1 = np.asarray(lk1, dtype=np.float32)
    lq2 = np.asarray(lq2, dtype=np.float32)
    lk2 = np.asarray(lk2, dtype=np.float32)

    lam1 = np.exp(np.sum(lq1 * lk1, dtype=np.float32))
    lam2 = np.exp(np.sum(lq2 * lk2, dtype=np.float32))
    lam = float(lam1 - lam2 + LAMBDA_INIT)

    nc = _get_program(lam)
    in_maps = _prep_inputs(x, Wq, Wk, Wv, Wo)
    res = run_bass_kernel_spmd(nc, in_maps, core_ids=list(range(N_CORES)))

    out = np.empty((B, S, D), dtype=np.float32)
    for b in range(B):
        acc = res.results[4 * b]["outT"].astype(np.float32)
        for p in range(1, 4):
            acc += res.results[4 * b + p]["outT"].astype(np.float32)
        out[b] = acc.T
    return out


# revision 39
# speedup vs baseline: 1.1405x; 1.0025x over previous
"""DiffAttention Trainium2 kernel (v2).

Full inputs in, full output out. Sharding: 8 cores = (batch b in {0,1}) x
(head-pair p in {0..3}); each core handles one batch element and 2 of the 8
heads (= 4 of the 16 q/k half-heads, 2 v heads, 256 of the 1024 o columns).
Out-projection is column-split: each core produces a full (S, D) partial of
o @ Wo.T restricted to its o columns; host sums the 4 partials per batch.

v2 changes over the baseline:
- q/k/v projections run in fp8e4m3 with DoubleRow perf mode (0.5 cyc/row,
  256-deep contraction). Accuracy is restored with a 3-chain residual
  decomposition: with x~=(x8+r8)/b and W~=(W8+rW8)/a (operands pre-scaled
  into fp8's sweet range host-side), x@W = [x8@W8 + (x8@rW8 + r8@W8)]/(ab),
  dropping the O(eps^2) r8@rW8 term. The main chain is 4 DoubleRow matmuls
  pairing kt tiles of x8/W8; the correction is 8 DoubleRow matmuls pairing
  (x8,r8) against (rW8,W8) per kt — together 12 DR matmuls (6 eff. rows)
  versus 8 fp16 matmuls, a 25% PE saving at ~2e-3 overall rel err.
- Projections are emitted per 512-column S-chunk and software-pipelined into
  the attention strip loop (strip s consumes chunk s; chunk s+1's projection
  groups are interleaved between strip s's kt iterations). This overlaps the
  exp-heavy ACT phase with projection PE work and fills PE dependency
  bubbles with independent projection matmuls.
- Evacuation copies use nc.any so the Tile scheduler balances DVE/ACT.
- Input DMAs avoid the ACT queue (exp is the second-busiest engine);
  weight/x DMAs are split fine-grained so the first projection starts ~3us
  in; output DMA is split in half per strip to shorten the kernel tail.

Attention math per head h (half-heads e0=2h, e1=2h+1), per q row:
  u_i = exp(s_i) @ v   (unnormalized), sum_i = exp(s_i) @ 1  (fused: rhs=[v|1])
  o   = u0/sum0 - lam * u1/sum1
  o   = o * rsqrt(mean(o^2)+eps) * (1-lam_init);   out = o @ Wo.T
Scores are computed transposed (keys on partitions, q on free dim) so the
exp'd tiles feed the PV matmul directly as the stationary operand. rsqrt is
Newton-Raphson on the DVE (fast-inverse-sqrt seed), batched per (strip, head),
keeping the ACT engine exp-only. PSUM banks: scores 2x2, u 2x1, and a shared
2x1 pool for projection chunks, o^T transposes and the out-projection.
"""

import math

import numpy as np

B = 2
S = 2048
D = 1024
H = 8
HD = 64  # half-head dim
LAMBDA_INIT = 0.8 - 0.6 * math.exp(-0.3 * 6)
EPS = 1e-5

N_CORES = 8
KT = D // 128       # 8 contraction tiles for projections
ST = S // 128       # 16 sequence tiles
NSTRIP = S // 512   # 4 q strips

BX = 16.0     # x pre-quantization scale
AQ = 1024.0   # Wq.T * hd^-0.5 scale
AK = 128.0    # Wk.T scale
AV = 128.0    # Wv.T scale


def _build_program(lam: float):
    import concourse.bass as bass
    import concourse.tile as tile
    from concourse import bacc, mybir
    from concourse.masks import make_identity

    f16 = mybir.dt.float16
    f32 = mybir.dt.float32
    f8 = mybir.dt.float8e4
    u32 = mybir.dt.uint32
    AF = mybir.ActivationFunctionType
    OP = mybir.AluOpType
    DR = mybir.MatmulPerfMode.DoubleRow

    nc = bacc.Bacc("TRN2", target_bir_lowering=False, debug=False,
                   num_devices=N_CORES)

    # packed fp8 inputs: var index 0 = x8 (scaled main), 1 = r8 (residual)
    # weights:           var index 0 = rW8 (residual), 1 = W8 (scaled main)
    xpack = nc.dram_tensor("xpack", (2, D, S), f8, kind="ExternalInput").ap()
    wqp = nc.dram_tensor("wqp", (2, D, 256), f8, kind="ExternalInput").ap()
    wkp = nc.dram_tensor("wkp", (2, D, 256), f8, kind="ExternalInput").ap()
    wvp = nc.dram_tensor("wvp", (2, D, 256), f8, kind="ExternalInput").ap()
    woT = nc.dram_tensor("woT", (256, D), f16, kind="ExternalInput").ap()
    outT = nc.dram_tensor("outT", (D, S), f16, kind="ExternalOutput").ap()

    with tile.TileContext(nc) as tc:
        with (
            tc.tile_pool(name="const", bufs=1) as cpool,
            tc.tile_pool(name="persist", bufs=1) as pp,
        ):
            ident = cpool.tile([128, 128], f16, tag="ident")
            make_identity(nc, ident)
            # mask[p, f] = 1 if p <= f else 0 (keys on partitions, q on free)
            maskt = cpool.tile([128, 128], f16, tag="maskt")
            nc.gpsimd.memset(maskt, 1.0)
            nc.gpsimd.affine_select(
                out=maskt, in_=maskt, compare_op=OP.is_ge, fill=0.0,
                base=0, pattern=[[1, 128]], channel_multiplier=-1,
            )

            # constants for Newton-Raphson rsqrt (fast-inverse-sqrt seed)
            magic_c = cpool.tile([128, 8], u32, tag="magic_c")
            nc.gpsimd.memset(magic_c, 0x5F3759DF)
            one_u = cpool.tile([128, 8], u32, tag="one_u")
            nc.gpsimd.memset(one_u, 1)

            wo_sb = pp.tile([128, 2, D], f16, tag="wo_sb")
            qT_sb = pp.tile([128, 2, S], f16, tag="qT_sb")
            kT_sb = pp.tile([128, 2, S], f16, tag="kT_sb")
            # v with a ones column appended per head: [v_h0 | 1 | v_h1 | 1]
            v_sb = pp.tile([128, ST, 258], f16, tag="v_sb")
            nc.vector.memset(v_sb[:, :, 128:129], 1.0)
            nc.vector.memset(v_sb[:, :, 257:258], 1.0)
            oT_sb = pp.tile([128, 2, S], f16, tag="oT_sb")

            # fp8 packed inputs stay resident for the whole kernel
            xp_sb = pp.tile([128, 2, KT, S], f8, tag="xp_sb")
            wq_sb = pp.tile([128, 2, KT, 256], f8, tag="wq_sb")
            wk_sb = pp.tile([128, 2, KT, 256], f8, tag="wk_sb")
            wv_sb = pp.tile([128, 2, KT, 256], f8, tag="wv_sb")

            xp_r = xpack.rearrange("v (kt p) s -> p v kt s", p=128)
            wq_r = wqp.rearrange("v (kt p) m -> p v kt m", p=128)
            wk_r = wkp.rearrange("v (kt p) m -> p v kt m", p=128)
            wv_r = wvp.rearrange("v (kt p) m -> p v kt m", p=128)

            # weight DMAs on Pool queue (ACT must stay exp-only), x chunks on
            # SP, main (v=0) halves before residual (v=1) so the first
            # projection matmuls can start early.
            nc.gpsimd.dma_start(wq_sb[:, 1, 0:2, :], wq_r[:, 1, 0:2, :])
            nc.gpsimd.dma_start(wq_sb[:, 1, 2:8, :], wq_r[:, 1, 2:8, :])
            nc.gpsimd.dma_start(wk_sb[:, 1, :, :], wk_r[:, 1, :, :])
            nc.gpsimd.dma_start(wq_sb[:, 0, :, :], wq_r[:, 0, :, :])
            nc.gpsimd.dma_start(wk_sb[:, 0, :, :], wk_r[:, 0, :, :])
            nc.sync.dma_start(xp_sb[:, 0, 0:4, 0:512], xp_r[:, 0, 0:4, 0:512])
            nc.sync.dma_start(xp_sb[:, 0, 4:8, 0:512], xp_r[:, 0, 4:8, 0:512])
            nc.sync.dma_start(xp_sb[:, 1, :, 0:512], xp_r[:, 1, :, 0:512])
            for v in (1, 0):
                nc.gpsimd.dma_start(wv_sb[:, v, :, :], wv_r[:, v, :, :])
            for c in range(1, 4):
                sl = slice(c * 512, (c + 1) * 512)
                nc.sync.dma_start(xp_sb[:, 0, :, sl], xp_r[:, 0, :, sl])
                nc.sync.dma_start(xp_sb[:, 1, :, sl], xp_r[:, 1, :, sl])
            nc.gpsimd.dma_start(
                wo_sb[:, :, :],
                woT.rearrange("(kt p) n -> p kt n", p=128)[:, :, :])

            with (
                tc.tile_pool(name="e0p", bufs=17) as e0pool,
                tc.tile_pool(name="e1p", bufs=17) as e1pool,
                tc.tile_pool(name="ps_s", bufs=2, space="PSUM") as ps_s,
                tc.tile_pool(name="ps_u", bufs=2, space="PSUM") as ps_u,
                tc.tile_pool(name="po", bufs=2, space="PSUM") as po,
                tc.tile_pool(name="nrm", bufs=8) as nrm,
                tc.tile_pool(name="nrm_big", bufs=3) as nrm_big,
                tc.tile_pool(name="osb", bufs=3) as osb,
                tc.tile_pool(name="out_sb", bufs=2) as out_pool,
            ):
                epools = {0: e0pool, 1: e1pool}
                outT_r = outT.rearrange("(mt p) s -> p mt s", p=128)

                # ---------------- fp8 projection groups ----------------
                def emit_proj_qk(w_sb, dst_sb, mt, c, inv_ab):
                    ps = po.tile([128, 512], f32, tag="po")
                    msl = slice(mt * 128, (mt + 1) * 128)
                    csl = slice(c * 512, (c + 1) * 512)
                    for j in range(4):
                        nc.tensor.matmul(
                            ps[:],
                            lhsT=w_sb[:, 1, 2 * j:2 * j + 2, msl],
                            rhs=xp_sb[:, 0, 2 * j:2 * j + 2, csl],
                            start=(j == 0), stop=False, perf_mode=DR,
                        )
                    for kt in range(KT):
                        nc.tensor.matmul(
                            ps[:],
                            lhsT=w_sb[:, :, kt, msl],
                            rhs=xp_sb[:, :, kt, csl],
                            start=False, stop=(kt == KT - 1), perf_mode=DR,
                        )
                    nc.vector.tensor_scalar(dst_sb[:, mt, csl], ps[:], inv_ab,
                                            None, OP.mult)

                def emit_proj_v(st, inv_ab):
                    ps = po.tile([128, 512], f32, tag="po")
                    ssl = slice(st * 128, (st + 1) * 128)
                    for j in range(4):
                        nc.tensor.matmul(
                            ps[:, 0:256],
                            lhsT=xp_sb[:, 0, 2 * j:2 * j + 2, ssl],
                            rhs=wv_sb[:, 1, 2 * j:2 * j + 2, :],
                            start=(j == 0), stop=False, perf_mode=DR,
                        )
                    for kt in range(KT):
                        nc.tensor.matmul(
                            ps[:, 0:256],
                            lhsT=xp_sb[:, :, kt, ssl],
                            rhs=wv_sb[:, :, kt, :],
                            start=False, stop=(kt == KT - 1), perf_mode=DR,
                        )
                    nc.vector.tensor_scalar(v_sb[:, st, 0:128], ps[:, 0:128],
                                            inv_ab, None, OP.mult)
                    nc.vector.tensor_scalar(v_sb[:, st, 129:257],
                                            ps[:, 128:256], inv_ab, None,
                                            OP.mult)

                def proj_chunk_groups(c):
                    gs = []
                    for mt in range(2):
                        gs.append(lambda mt=mt: emit_proj_qk(
                            wq_sb, qT_sb, mt, c, 1.0 / (AQ * BX)))
                        gs.append(lambda mt=mt: emit_proj_qk(
                            wk_sb, kT_sb, mt, c, 1.0 / (AK * BX)))
                    for st in range(4 * c, 4 * c + 4):
                        gs.append(lambda st=st: emit_proj_v(st, 1.0 / (AV * BX)))
                    return gs

                # ---------------- attention ----------------
                def pv_qtile(h, s, i, e_tiles, oq_s, ss_s):
                    qt = 4 * s + i
                    up = ps_u.tile([128, 258], f32, tag="u")
                    for kt in range(qt + 1):
                        c = i * 128
                        vh = v_sb[:, kt, 129 * h:129 * h + 129]
                        nc.tensor.matmul(
                            up[:, 0:129],
                            lhsT=e_tiles[kt][:, c:c + 128],
                            rhs=vh,
                            start=(kt == 0), stop=(kt == qt),
                        )
                        nc.tensor.matmul(
                            up[:, 129:258],
                            lhsT=e_tiles[kt][:, 512 + c:512 + c + 128],
                            rhs=vh,
                            start=False, stop=(kt == qt),
                            skip_group_check=True,
                        )
                    # normalized diff: oq = u0/s0 - lam*u1/s1 (per-partition)
                    inv0 = nrm.tile([128, 1], f32, tag="inv0")
                    nc.vector.reciprocal(inv0, up[:, 128:129])
                    inv1 = nrm.tile([128, 1], f32, tag="inv1")
                    nc.vector.reciprocal(inv1, up[:, 257:258])
                    t1 = nrm.tile([128, 128], f32, tag="t1")
                    nc.vector.tensor_scalar(t1, up[:, 129:257], inv1, lam,
                                            OP.mult, OP.mult)
                    oq = oq_s[:, i, :]
                    nc.vector.scalar_tensor_tensor(
                        oq, up[:, 0:128], inv0, t1, OP.mult, OP.subtract)
                    sq = nrm.tile([128, 128], f32, tag="sq")
                    nc.vector.scalar_tensor_tensor(
                        sq, oq, 1.0, oq, OP.bypass, OP.mult,
                        accum_out=ss_s[:, i:i + 1])

                def norm_tail(h, s, oq_s, ss_s, tpool=None, ttag="po"):
                    """Newton rsqrt over the strip's 4 q-tiles, then scale,
                    transpose and evacuate o^T."""
                    if tpool is None:
                        tpool = po
                    ms = nrm.tile([128, 4], f32, tag="ms")
                    il2 = 1.0 / ((1.0 - LAMBDA_INIT) ** 2)
                    nc.vector.tensor_scalar(ms, ss_s, il2 / 128.0, EPS * il2,
                                            OP.mult, OP.add)
                    y0 = nrm.tile([128, 4], u32, tag="y0")
                    nc.vector.tensor_tensor(y0, ms.bitcast(u32),
                                            one_u[:, 0:4],
                                            OP.logical_shift_right)
                    nc.vector.tensor_tensor(y0, magic_c[:, 0:4], y0,
                                            OP.subtract)
                    yf = y0.bitcast(f32)
                    t2 = nrm.tile([128, 4], f32, tag="t2")
                    r_all = nrm.tile([128, 4], f32, tag="r_all")
                    nc.vector.tensor_mul(t2, yf, yf)
                    nc.vector.tensor_mul(t2, t2, ms)
                    nc.vector.tensor_scalar(t2, t2, -0.5, 1.5, OP.mult, OP.add)
                    nc.vector.tensor_mul(r_all, yf, t2)
                    nc.vector.tensor_mul(t2, r_all, r_all)
                    nc.vector.tensor_mul(t2, t2, ms)
                    nc.vector.tensor_scalar(t2, t2, -0.5, 1.5, OP.mult, OP.add)
                    nc.vector.tensor_mul(r_all, r_all, t2)
                    for i in range(4):
                        qt = 4 * s + i
                        on = osb.tile([128, 128], f16, tag="on")
                        nc.vector.tensor_scalar(on, oq_s[:, i, :],
                                                r_all[:, i:i + 1], None,
                                                OP.mult)
                        pt = tpool.tile([128, 512], f16, tag=ttag)
                        nc.tensor.transpose(pt[:, 0:128], on, ident)
                        nc.vector.tensor_copy(
                            oT_sb[:, h, qt * 128:(qt + 1) * 128], pt[:, 0:128])

                def outproj_groups(s):
                    """8 filler groups (one per mt) computing strip s's
                    out-projection; interleaved into strip s+1."""
                    state = {}

                    def grp(mt):
                        if mt == 0:
                            ot = out_pool.tile([128, 8, 512], f16, tag="ot")
                            state["ot"] = ot
                        ot = state["ot"]
                        ps = po.tile([128, 512], f32, tag="po")
                        for kt in range(2):
                            nc.tensor.matmul(
                                ps[:],
                                lhsT=wo_sb[:, kt, mt * 128:(mt + 1) * 128],
                                rhs=oT_sb[:, kt, s * 512:(s + 1) * 512],
                                start=(kt == 0), stop=(kt == 1),
                            )
                        nc.vector.tensor_copy(ot[:, mt, :], ps[:])
                        if mt == 3:
                            nc.sync.dma_start(
                                outT_r[:, 0:4, s * 512:(s + 1) * 512],
                                ot[:, 0:4, :])
                        elif mt == 7:
                            nc.sync.dma_start(
                                outT_r[:, 4:8, s * 512:(s + 1) * 512],
                                ot[:, 4:8, :])

                    return [lambda mt=mt: grp(mt) for mt in range(8)]

                def outproj_last(s):
                    """Final strip: per-128-column sub-tile matmuls so 3/4 of
                    the PE work overlaps the final norm_tail; grouped evacs
                    alternate DVE/ACT; DMA per 2 mt."""
                    ot = out_pool.tile([128, 8, 512], f16, tag="ot")
                    for mt in range(8):
                        # borrow the idle scores-psum slots: 4 groups in
                        # flight instead of 2
                        if mt % 2 == 0:
                            ps = po.tile([128, 512], f32, tag="po")
                        else:
                            ps = ps_s.tile([128, 512], f32, tag="sc")
                        for sub in range(4):
                            for kt in range(2):
                                nc.tensor.matmul(
                                    ps[:, sub * 128:(sub + 1) * 128],
                                    lhsT=wo_sb[:, kt, mt * 128:(mt + 1) * 128],
                                    rhs=oT_sb[:, kt,
                                              s * 512 + sub * 128:
                                              s * 512 + (sub + 1) * 128],
                                    start=(kt == 0), stop=(kt == 1),
                                    skip_group_check=(sub > 0),
                                )
                        if mt % 2 == 1:
                            nc.scalar.copy(ot[:, mt, :], ps[:])
                        else:
                            nc.vector.tensor_copy(ot[:, mt, :], ps[:])
                        nc.sync.dma_start(
                            outT_r[:, mt:mt + 1, s * 512:(s + 1) * 512],
                            ot[:, mt:mt + 1, :])

                # chunk 0 projections up front
                for g in proj_chunk_groups(0):
                    g()

                for s in range(NSTRIP):
                    pending = []
                    if s > 0:
                        pending += outproj_groups(s - 1)
                    if s < NSTRIP - 1:
                        pending += proj_chunk_groups(s + 1)
                    niter = 2 * 4 * (s + 1)
                    total0 = len(pending)
                    emitted = 0
                    it = 0
                    for h in range(2):
                        e_tiles = {}
                        oq_s = nrm_big.tile([128, 4, 128], f32, tag="oq_s")
                        ss_s = nrm_big.tile([128, 4], f32, tag="ss_s")
                        for kt in range(4 * (s + 1)):
                            col0 = max(0, (kt - 4 * s) * 128)
                            pa = ps_s.tile([128, 1024], f32, tag="sc")
                            nc.tensor.matmul(
                                pa[:, col0:512],
                                lhsT=kT_sb[0:64, h, kt * 128:(kt + 1) * 128],
                                rhs=qT_sb[0:64, h, s * 512 + col0:(s + 1) * 512],
                                start=True, stop=True, tile_position=(0, 0),
                            )
                            nc.tensor.matmul(
                                pa[:, 512 + col0:1024],
                                lhsT=kT_sb[64:128, h, kt * 128:(kt + 1) * 128],
                                rhs=qT_sb[64:128, h, s * 512 + col0:(s + 1) * 512],
                                start=True, stop=True, tile_position=(64, 0),
                                skip_group_check=True,
                            )
                            ee = epools[h].tile([128, 1024], f16, tag="e")
                            # exp both half-heads in one ACT op (strided AP
                            # skips the invalid leading columns of each half)
                            nc.scalar.activation(
                                ee.rearrange("p (b c) -> p b c", b=2)[:, :, col0:512],
                                pa.rearrange("p (b c) -> p b c", b=2)[:, :, col0:512],
                                AF.Exp)
                            if kt >= 4 * s:
                                c = col0
                                nc.gpsimd.tensor_mul(ee[:, c:c + 128],
                                                     ee[:, c:c + 128], maskt)
                                nc.gpsimd.tensor_mul(ee[:, 512 + c:512 + c + 128],
                                                     ee[:, 512 + c:512 + c + 128],
                                                     maskt)
                            e_tiles[kt] = ee
                            if kt >= 4 * s:
                                pv_qtile(h, s, kt - 4 * s, e_tiles, oq_s, ss_s)
                            # interleave deferred work
                            it += 1
                            target = -(-total0 * it // niter)  # ceil
                            while pending and emitted < target:
                                pending.pop(0)()
                                emitted += 1
                        if s == NSTRIP - 1 and h == 1:
                            norm_tail(h, s, oq_s, ss_s, tpool=ps_s,
                                      ttag="sc")
                        else:
                            norm_tail(h, s, oq_s, ss_s)

                outproj_last(NSTRIP - 1)

    nc.compile()
    return nc


def _prep_inputs(x, Wq, Wk, Wv, Wo):
    """Build the 8 per-core input maps (host-side shard + fp8 packing)."""
    import ml_dtypes
    F8 = ml_dtypes.float8_e4m3
    f16 = np.float16

    def q8(a):
        return a.astype(F8)

    scale = HD ** -0.5
    # x packs are per batch: [D, 2, S] with var 0 = x8, var 1 = r8
    xpacks = []
    for b in range(B):
        xT = np.ascontiguousarray(x[b].T) * BX     # (D, S) fp32
        x8 = q8(xT)
        r8 = q8(xT - x8.astype(np.float32))
        xp = np.empty((2, D, S), dtype=F8)
        xp[0] = x8
        xp[1] = r8
        xpacks.append(xp)

    def wpack(W, a, pre=1.0):
        # W slice already (rows 256, D); computes pack of (W.T * pre) * a
        WT = np.ascontiguousarray(W.T) * (pre * a)  # (D, 256)
        W8 = q8(WT)
        rW8 = q8(WT - W8.astype(np.float32))
        wp = np.empty((2, D, 256), dtype=F8)
        wp[0] = rW8
        wp[1] = W8
        return wp

    in_maps = []
    for d in range(N_CORES):
        b, p = divmod(d, 4)
        r0 = 256 * p
        in_maps.append({
            "xpack": xpacks[b],
            "wqp": wpack(Wq[r0:r0 + 256, :], AQ, pre=scale),
            "wkp": wpack(Wk[r0:r0 + 256, :], AK),
            "wvp": wpack(Wv[r0:r0 + 256, :], AV),
            "woT": np.ascontiguousarray(Wo[:, r0:r0 + 256].T).astype(f16),
        })
    return in_maps


_CACHED = {}


def _get_program(lam: float):
    # the program depends on inputs only through lam
    key = round(float(lam), 9)
    if key not in _CACHED:
        _CACHED[key] = _build_program(float(lam))
    return _CACHED[key]


def kernel(x, Wq, Wk, Wv, Wo, lq1, lk1, lq2, lk2):
    from concourse.bass_utils import run_bass_kernel_spmd

    x = np.asarray(x, dtype=np.float32)
    Wq = np.asarray(Wq, dtype=np.float32)
    Wk = np.asarray(Wk, dtype=np.float32)
    Wv = np.asarray(Wv, dtype=np.float32)
    Wo = np.asarray(Wo, dtype=np.float32)
    lq1 = np.asarray(lq1, dtype=np.float32)
    lk1 = np.asarray(lk1, dtype=np.float32)
    lq2 = np.asarray(lq2, dtype=np.float32)
    lk2 = np.asarray(lk2, dtype=np.float32)

    lam1 = np.exp(np.sum(lq1 * lk1, dtype=np.float32))
    lam2 = np.exp(np.sum(lq2 * lk2, dtype=np.float32))
    lam = float(lam1 - lam2 + LAMBDA_INIT)

    nc = _get_program(lam)
    in_maps = _prep_inputs(x, Wq, Wk, Wv, Wo)
    res = run_bass_kernel_spmd(nc, in_maps, core_ids=list(range(N_CORES)))

    out = np.empty((B, S, D), dtype=np.float32)
    for b in range(B):
        acc = res.results[4 * b]["outT"].astype(np.float32)
        for p in range(1, 4):
            acc += res.results[4 * b + p]["outT"].astype(np.float32)
        out[b] = acc.T
    return out


# revision 46
# speedup vs baseline: 1.1491x; 1.0075x over previous
"""DiffAttention Trainium2 kernel (v2).

Full inputs in, full output out. Sharding: 8 cores = (batch b in {0,1}) x
(head-pair p in {0..3}); each core handles one batch element and 2 of the 8
heads (= 4 of the 16 q/k half-heads, 2 v heads, 256 of the 1024 o columns).
Out-projection is column-split: each core produces a full (S, D) partial of
o @ Wo.T restricted to its o columns; host sums the 4 partials per batch.

v2 changes over the baseline:
- q/k/v projections run in fp8e4m3 with DoubleRow perf mode (0.5 cyc/row,
  256-deep contraction). Accuracy is restored with a 3-chain residual
  decomposition: with x~=(x8+r8)/b and W~=(W8+rW8)/a (operands pre-scaled
  into fp8's sweet range host-side), x@W = [x8@W8 + (x8@rW8 + r8@W8)]/(ab),
  dropping the O(eps^2) r8@rW8 term. The main chain is 4 DoubleRow matmuls
  pairing kt tiles of x8/W8; the correction is 8 DoubleRow matmuls pairing
  (x8,r8) against (rW8,W8) per kt — together 12 DR matmuls (6 eff. rows)
  versus 8 fp16 matmuls, a 25% PE saving at ~2e-3 overall rel err.
- Projections are emitted per 512-column S-chunk and software-pipelined into
  the attention strip loop (strip s consumes chunk s; chunk s+1's projection
  groups are interleaved between strip s's kt iterations). This overlaps the
  exp-heavy ACT phase with projection PE work and fills PE dependency
  bubbles with independent projection matmuls.
- Evacuation copies use nc.any so the Tile scheduler balances DVE/ACT.
- Input DMAs avoid the ACT queue (exp is the second-busiest engine);
  weight/x DMAs are split fine-grained so the first projection starts ~3us
  in; output DMA is split in half per strip to shorten the kernel tail.

Attention math per head h (half-heads e0=2h, e1=2h+1), per q row:
  u_i = exp(s_i) @ v   (unnormalized), sum_i = exp(s_i) @ 1  (fused: rhs=[v|1])
  o   = u0/sum0 - lam * u1/sum1
  o   = o * rsqrt(mean(o^2)+eps) * (1-lam_init);   out = o @ Wo.T
Scores are computed transposed (keys on partitions, q on free dim) so the
exp'd tiles feed the PV matmul directly as the stationary operand. rsqrt is
Newton-Raphson on the DVE (fast-inverse-sqrt seed), batched per (strip, head),
keeping the ACT engine exp-only. PSUM banks: scores 2x2, u 2x1, and a shared
2x1 pool for projection chunks, o^T transposes and the out-projection.
"""

import math

import numpy as np

B = 2
S = 2048
D = 1024
H = 8
HD = 64  # half-head dim
LAMBDA_INIT = 0.8 - 0.6 * math.exp(-0.3 * 6)
EPS = 1e-5

N_CORES = 8
KT = D // 128       # 8 contraction tiles for projections
ST = S // 128       # 16 sequence tiles
NSTRIP = S // 512   # 4 q strips

BX = 16.0     # x pre-quantization scale
AQ = 1024.0   # Wq.T * hd^-0.5 scale
AK = 128.0    # Wk.T scale
AV = 128.0    # Wv.T scale


def _build_program(lam: float):
    import concourse.bass as bass
    import concourse.tile as tile
    from concourse import bacc, mybir
    from concourse.masks import make_identity

    f16 = mybir.dt.float16
    f32 = mybir.dt.float32
    f8 = mybir.dt.float8e4
    u32 = mybir.dt.uint32
    AF = mybir.ActivationFunctionType
    OP = mybir.AluOpType
    DR = mybir.MatmulPerfMode.DoubleRow

    nc = bacc.Bacc("TRN2", target_bir_lowering=False, debug=False,
                   num_devices=N_CORES)

    # packed fp8 inputs: var index 0 = x8 (scaled main), 1 = r8 (residual)
    # weights:           var index 0 = rW8 (residual), 1 = W8 (scaled main)
    xpack = nc.dram_tensor("xpack", (2, D, S), f8, kind="ExternalInput").ap()
    wqp = nc.dram_tensor("wqp", (2, D, 256), f8, kind="ExternalInput").ap()
    wkp = nc.dram_tensor("wkp", (2, D, 256), f8, kind="ExternalInput").ap()
    wvp = nc.dram_tensor("wvp", (2, D, 256), f8, kind="ExternalInput").ap()
    woT = nc.dram_tensor("woT", (256, D), f16, kind="ExternalInput").ap()
    outT = nc.dram_tensor("outT", (D, S), f16, kind="ExternalOutput").ap()

    with tile.TileContext(nc) as tc:
        with (
            tc.tile_pool(name="const", bufs=1) as cpool,
            tc.tile_pool(name="persist", bufs=1) as pp,
        ):
            ident = cpool.tile([128, 128], f16, tag="ident")
            make_identity(nc, ident)
            # mask[p, f] = 1 if p <= f else 0 (keys on partitions, q on free)
            maskt = cpool.tile([128, 128], f16, tag="maskt")
            nc.gpsimd.memset(maskt, 1.0)
            nc.gpsimd.affine_select(
                out=maskt, in_=maskt, compare_op=OP.is_ge, fill=0.0,
                base=0, pattern=[[1, 128]], channel_multiplier=-1,
            )

            # constants for Newton-Raphson rsqrt (fast-inverse-sqrt seed)
            magic_c = cpool.tile([128, 8], u32, tag="magic_c")
            nc.gpsimd.memset(magic_c, 0x5F3759DF)
            one_u = cpool.tile([128, 8], u32, tag="one_u")
            nc.gpsimd.memset(one_u, 1)

            wo_sb = pp.tile([128, 2, D], f16, tag="wo_sb")
            qT_sb = pp.tile([128, 2, S], f16, tag="qT_sb")
            kT_sb = pp.tile([128, 2, S], f16, tag="kT_sb")
            # v with a ones column appended per head: [v_h0 | 1 | v_h1 | 1]
            v_sb = pp.tile([128, ST, 258], f16, tag="v_sb")
            nc.vector.memset(v_sb[:, :, 128:129], 1.0)
            nc.vector.memset(v_sb[:, :, 257:258], 1.0)
            oT_sb = pp.tile([128, 2, S], f16, tag="oT_sb")

            # fp8 packed inputs stay resident for the whole kernel
            xp_sb = pp.tile([128, 2, KT, S], f8, tag="xp_sb")
            wq_sb = pp.tile([128, 2, KT, 256], f8, tag="wq_sb")
            wk_sb = pp.tile([128, 2, KT, 256], f8, tag="wk_sb")
            wv_sb = pp.tile([128, 2, KT, 256], f8, tag="wv_sb")

            xp_r = xpack.rearrange("v (kt p) s -> p v kt s", p=128)
            wq_r = wqp.rearrange("v (kt p) m -> p v kt m", p=128)
            wk_r = wkp.rearrange("v (kt p) m -> p v kt m", p=128)
            wv_r = wvp.rearrange("v (kt p) m -> p v kt m", p=128)

            # weight DMAs on Pool queue (ACT must stay exp-only), x chunks on
            # SP, main (v=0) halves before residual (v=1) so the first
            # projection matmuls can start early.
            nc.gpsimd.dma_start(wq_sb[:, 1, 0:2, :], wq_r[:, 1, 0:2, :])
            nc.gpsimd.dma_start(wq_sb[:, 1, 2:4, :], wq_r[:, 1, 2:4, :])
            nc.gpsimd.dma_start(wq_sb[:, 1, 4:8, :], wq_r[:, 1, 4:8, :])
            nc.gpsimd.dma_start(wk_sb[:, 1, :, :], wk_r[:, 1, :, :])
            nc.gpsimd.dma_start(wq_sb[:, 0, :, :], wq_r[:, 0, :, :])
            nc.gpsimd.dma_start(wk_sb[:, 0, :, :], wk_r[:, 0, :, :])
            nc.sync.dma_start(xp_sb[:, 0, 0:4, 0:512], xp_r[:, 0, 0:4, 0:512])
            nc.sync.dma_start(xp_sb[:, 0, 4:8, 0:512], xp_r[:, 0, 4:8, 0:512])
            nc.sync.dma_start(xp_sb[:, 1, :, 0:512], xp_r[:, 1, :, 0:512])
            for v in (1, 0):
                nc.gpsimd.dma_start(wv_sb[:, v, :, :], wv_r[:, v, :, :])
            for c in range(1, 4):
                sl = slice(c * 512, (c + 1) * 512)
                nc.sync.dma_start(xp_sb[:, 0, :, sl], xp_r[:, 0, :, sl])
                nc.sync.dma_start(xp_sb[:, 1, :, sl], xp_r[:, 1, :, sl])
            nc.gpsimd.dma_start(
                wo_sb[:, :, :],
                woT.rearrange("(kt p) n -> p kt n", p=128)[:, :, :])

            with (
                tc.tile_pool(name="e0p", bufs=16) as e0pool,
                tc.tile_pool(name="e1p", bufs=16) as e1pool,
                tc.tile_pool(name="ps_s", bufs=2, space="PSUM") as ps_s,
                tc.tile_pool(name="ps_u", bufs=2, space="PSUM") as ps_u,
                tc.tile_pool(name="po", bufs=2, space="PSUM") as po,
                tc.tile_pool(name="nrm", bufs=8) as nrm,
                tc.tile_pool(name="nrm_big", bufs=3) as nrm_big,
                tc.tile_pool(name="osb", bufs=3) as osb,
                tc.tile_pool(name="out_sb", bufs=2) as out_pool,
                tc.tile_pool(name="pre3", bufs=8) as pre_pool,
            ):
                epools = {0: e0pool, 1: e1pool}
                outT_r = outT.rearrange("(mt p) s -> p mt s", p=128)

                # ---------------- fp8 projection groups ----------------
                def proj_qk_main(w_sb, mt, c):
                    ps = po.tile([128, 512], f32, tag="po")
                    msl = slice(mt * 128, (mt + 1) * 128)
                    csl = slice(c * 512, (c + 1) * 512)
                    for j in range(4):
                        nc.tensor.matmul(
                            ps[:],
                            lhsT=w_sb[:, 1, 2 * j:2 * j + 2, msl],
                            rhs=xp_sb[:, 0, 2 * j:2 * j + 2, csl],
                            start=(j == 0), stop=False, perf_mode=DR,
                        )
                    return ps

                def proj_qk_corr(ps, w_sb, dst_sb, mt, c, inv_ab):
                    msl = slice(mt * 128, (mt + 1) * 128)
                    csl = slice(c * 512, (c + 1) * 512)
                    for kt in range(KT):
                        nc.tensor.matmul(
                            ps[:],
                            lhsT=w_sb[:, :, kt, msl],
                            rhs=xp_sb[:, :, kt, csl],
                            start=False, stop=(kt == KT - 1), perf_mode=DR,
                        )
                    nc.vector.tensor_scalar(dst_sb[:, mt, csl], ps[:], inv_ab,
                                            None, OP.mult)

                def emit_proj_qk(w_sb, dst_sb, mt, c, inv_ab):
                    ps = proj_qk_main(w_sb, mt, c)
                    proj_qk_corr(ps, w_sb, dst_sb, mt, c, inv_ab)

                def emit_proj_v(st, inv_ab):
                    ps = po.tile([128, 512], f32, tag="po")
                    ssl = slice(st * 128, (st + 1) * 128)
                    for j in range(4):
                        nc.tensor.matmul(
                            ps[:, 0:256],
                            lhsT=xp_sb[:, 0, 2 * j:2 * j + 2, ssl],
                            rhs=wv_sb[:, 1, 2 * j:2 * j + 2, :],
                            start=(j == 0), stop=False, perf_mode=DR,
                        )
                    for kt in range(KT):
                        nc.tensor.matmul(
                            ps[:, 0:256],
                            lhsT=xp_sb[:, :, kt, ssl],
                            rhs=wv_sb[:, :, kt, :],
                            start=False, stop=(kt == KT - 1), perf_mode=DR,
                        )
                    nc.vector.tensor_scalar(v_sb[:, st, 0:128], ps[:, 0:128],
                                            inv_ab, None, OP.mult)
                    nc.vector.tensor_scalar(v_sb[:, st, 129:257],
                                            ps[:, 128:256], inv_ab, None,
                                            OP.mult)

                def proj_q_groups(c):
                    return [lambda mt=mt: emit_proj_qk(
                        wq_sb, qT_sb, mt, c, 1.0 / (AQ * BX))
                        for mt in range(2)]

                def proj_kv_groups(c):
                    gs = [lambda mt=mt: emit_proj_qk(
                        wk_sb, kT_sb, mt, c, 1.0 / (AK * BX))
                        for mt in range(2)]
                    for st in range(4 * c, 4 * c + 4):
                        gs.append(lambda st=st: emit_proj_v(st, 1.0 / (AV * BX)))
                    return gs

                def proj_chunk_groups(c):
                    gs = []
                    qg = proj_q_groups(c)
                    kg = proj_kv_groups(c)
                    gs += [qg[0], kg[0], qg[1], kg[1]] + kg[2:]
                    return gs

                # ---------------- attention ----------------
                def pv_qtile(h, s, i, e_tiles, oq_s, ss_s):
                    qt = 4 * s + i
                    up = ps_u.tile([128, 258], f32, tag="u")
                    for kt in range(qt + 1):
                        c = i * 128
                        vh = v_sb[:, kt, 129 * h:129 * h + 129]
                        nc.tensor.matmul(
                            up[:, 0:129],
                            lhsT=e_tiles[kt][:, c:c + 128],
                            rhs=vh,
                            start=(kt == 0), stop=(kt == qt),
                        )
                        nc.tensor.matmul(
                            up[:, 129:258],
                            lhsT=e_tiles[kt][:, 512 + c:512 + c + 128],
                            rhs=vh,
                            start=False, stop=(kt == qt),
                            skip_group_check=True,
                        )
                    # normalized diff: oq = u0/s0 - lam*u1/s1 (per-partition)
                    inv0 = nrm.tile([128, 1], f32, tag="inv0")
                    nc.vector.reciprocal(inv0, up[:, 128:129])
                    inv1 = nrm.tile([128, 1], f32, tag="inv1")
                    nc.vector.reciprocal(inv1, up[:, 257:258])
                    t1 = nrm.tile([128, 128], f32, tag="t1")
                    nc.vector.tensor_scalar(t1, up[:, 129:257], inv1, lam,
                                            OP.mult, OP.mult)
                    oq = oq_s[:, i, :]
                    nc.vector.scalar_tensor_tensor(
                        oq, up[:, 0:128], inv0, t1, OP.mult, OP.subtract)
                    sq = nrm.tile([128, 128], f32, tag="sq")
                    nc.vector.scalar_tensor_tensor(
                        sq, oq, 1.0, oq, OP.bypass, OP.mult,
                        accum_out=ss_s[:, i:i + 1])

                def norm_tail(h, s, oq_s, ss_s, tpool=None, ttag="po"):
                    """Newton rsqrt over the strip's 4 q-tiles, then scale,
                    transpose and evacuate o^T."""
                    if tpool is None:
                        tpool = po
                    ms = nrm.tile([128, 4], f32, tag="ms")
                    il2 = 1.0 / ((1.0 - LAMBDA_INIT) ** 2)
                    nc.vector.tensor_scalar(ms, ss_s, il2 / 128.0, EPS * il2,
                                            OP.mult, OP.add)
                    y0 = nrm.tile([128, 4], u32, tag="y0")
                    nc.vector.tensor_tensor(y0, ms.bitcast(u32),
                                            one_u[:, 0:4],
                                            OP.logical_shift_right)
                    nc.vector.tensor_tensor(y0, magic_c[:, 0:4], y0,
                                            OP.subtract)
                    yf = y0.bitcast(f32)
                    t2 = nrm.tile([128, 4], f32, tag="t2")
                    r_all = nrm.tile([128, 4], f32, tag="r_all")
                    nc.vector.tensor_mul(t2, yf, yf)
                    nc.vector.tensor_mul(t2, t2, ms)
                    nc.vector.tensor_scalar(t2, t2, -0.5, 1.5, OP.mult, OP.add)
                    nc.vector.tensor_mul(r_all, yf, t2)
                    nc.vector.tensor_mul(t2, r_all, r_all)
                    nc.vector.tensor_mul(t2, t2, ms)
                    nc.vector.tensor_scalar(t2, t2, -0.5, 1.5, OP.mult, OP.add)
                    nc.vector.tensor_mul(r_all, r_all, t2)
                    for i in range(4):
                        qt = 4 * s + i
                        on = osb.tile([128, 128], f16, tag="on")
                        nc.vector.tensor_scalar(on, oq_s[:, i, :],
                                                r_all[:, i:i + 1], None,
                                                OP.mult)
                        pt = tpool.tile([128, 512], f16, tag=ttag)
                        nc.tensor.transpose(pt[:, 0:128], on, ident)
                        nc.vector.tensor_copy(
                            oT_sb[:, h, qt * 128:(qt + 1) * 128], pt[:, 0:128])

                def outproj_groups(s):
                    """8 filler groups (one per mt) computing strip s's
                    out-projection; interleaved into strip s+1."""
                    state = {}

                    def grp(mt):
                        if mt == 0:
                            ot = out_pool.tile([128, 8, 512], f16, tag="ot")
                            state["ot"] = ot
                        ot = state["ot"]
                        ps = po.tile([128, 512], f32, tag="po")
                        for kt in range(2):
                            nc.tensor.matmul(
                                ps[:],
                                lhsT=wo_sb[:, kt, mt * 128:(mt + 1) * 128],
                                rhs=oT_sb[:, kt, s * 512:(s + 1) * 512],
                                start=(kt == 0), stop=(kt == 1),
                            )
                        nc.vector.tensor_copy(ot[:, mt, :], ps[:])
                        if mt == 3:
                            nc.sync.dma_start(
                                outT_r[:, 0:4, s * 512:(s + 1) * 512],
                                ot[:, 0:4, :])
                        elif mt == 7:
                            nc.sync.dma_start(
                                outT_r[:, 4:8, s * 512:(s + 1) * 512],
                                ot[:, 4:8, :])

                    return [lambda mt=mt: grp(mt) for mt in range(8)]

                def outproj_last(s):
                    """Final strip: per-128-column sub-tile matmuls so 3/4 of
                    the PE work overlaps the final norm_tail; grouped evacs
                    alternate DVE/ACT; DMA per 2 mt."""
                    ot = out_pool.tile([128, 8, 512], f16, tag="ot")
                    for mt in range(8):
                        # borrow the idle scores-psum slots: 4 groups in
                        # flight instead of 2
                        if mt % 2 == 0:
                            ps = po.tile([128, 512], f32, tag="po")
                        else:
                            ps = ps_s.tile([128, 512], f32, tag="sc")
                        for sub in range(4):
                            for kt in range(2):
                                nc.tensor.matmul(
                                    ps[:, sub * 128:(sub + 1) * 128],
                                    lhsT=wo_sb[:, kt, mt * 128:(mt + 1) * 128],
                                    rhs=oT_sb[:, kt,
                                              s * 512 + sub * 128:
                                              s * 512 + (sub + 1) * 128],
                                    start=(kt == 0), stop=(kt == 1),
                                    skip_group_check=(sub > 0),
                                )
                        if mt % 2 == 1:
                            nc.scalar.copy(ot[:, mt, :], ps[:])
                            q = nc.sync if mt % 4 == 1 else nc.gpsimd
                            q.dma_start(
                                outT_r[:, mt - 1:mt + 1,
                                       s * 512:(s + 1) * 512],
                                ot[:, mt - 1:mt + 1, :])
                        else:
                            nc.vector.tensor_copy(ot[:, mt, :], ps[:])

                pre_tiles = {}

                def early_sc_group(h, kt):
                    sl = 3 * 512  # strip 3 q columns
                    pa = ps_s.tile([128, 1024], f32, tag="sc")
                    nc.tensor.matmul(
                        pa[:, 0:512],
                        lhsT=kT_sb[0:64, h, kt * 128:(kt + 1) * 128],
                        rhs=qT_sb[0:64, h, sl:sl + 512],
                        start=True, stop=True, tile_position=(0, 0),
                    )
                    nc.tensor.matmul(
                        pa[:, 512:1024],
                        lhsT=kT_sb[64:128, h, kt * 128:(kt + 1) * 128],
                        rhs=qT_sb[64:128, h, sl:sl + 512],
                        start=True, stop=True, tile_position=(64, 0),
                        skip_group_check=True,
                    )
                    ee = pre_pool.tile([128, 1024], f16, tag="pe3")
                    nc.scalar.activation(ee, pa, AF.Exp)
                    pre_tiles[(h, kt)] = ee

                NPRE = 4  # strip-3 kt tiles precomputed per head

                # chunk 0 projections up front; the first two groups'
                # main chains run before any correction chain so the PE
                # isn't blocked in-order on the residual (v=1) DMAs
                ps_q0 = proj_qk_main(wq_sb, 0, 0)
                ps_k0 = proj_qk_main(wk_sb, 0, 0)
                proj_qk_corr(ps_q0, wq_sb, qT_sb, 0, 0, 1.0 / (AQ * BX))
                proj_qk_corr(ps_k0, wk_sb, kT_sb, 0, 0, 1.0 / (AK * BX))
                emit_proj_qk(wq_sb, qT_sb, 1, 0, 1.0 / (AQ * BX))
                emit_proj_qk(wk_sb, kT_sb, 1, 0, 1.0 / (AK * BX))
                for st in range(4):
                    emit_proj_v(st, 1.0 / (AV * BX))

                for s in range(NSTRIP):
                    pending = []
                    if s == 0:
                        pending += proj_chunk_groups(1)
                        pending += proj_q_groups(2) + proj_q_groups(3)
                    elif s == 1:
                        early = [lambda h=h, kt=kt: early_sc_group(h, kt)
                                 for h in range(2) for kt in range(NPRE)]
                        rest = outproj_groups(0) + proj_kv_groups(2)
                        pending = [g for pair in zip(early, rest)
                                   for g in pair]
                        pending += early[len(rest):] + rest[len(early):]
                    elif s == 2:
                        pending += outproj_groups(1) + proj_kv_groups(3)
                    else:
                        pending += outproj_groups(2)
                    niter = 2 * 4 * (s + 1)
                    total0 = len(pending)
                    emitted = 0
                    it = 0
                    for h in range(2):
                        e_tiles = {}
                        oq_s = nrm_big.tile([128, 4, 128], f32, tag="oq_s")
                        ss_s = nrm_big.tile([128, 4], f32, tag="ss_s")
                        for kt in range(4 * (s + 1)):
                            if s == NSTRIP - 1 and kt < NPRE:
                                e_tiles[kt] = pre_tiles[(h, kt)]
                                it += 1
                                target = -(-total0 * it // niter)  # ceil
                                while pending and emitted < target:
                                    pending.pop(0)()
                                    emitted += 1
                                continue
                            col0 = max(0, (kt - 4 * s) * 128)
                            pa = ps_s.tile([128, 1024], f32, tag="sc")
                            nc.tensor.matmul(
                                pa[:, col0:512],
                                lhsT=kT_sb[0:64, h, kt * 128:(kt + 1) * 128],
                                rhs=qT_sb[0:64, h, s * 512 + col0:(s + 1) * 512],
                                start=True, stop=True, tile_position=(0, 0),
                            )
                            nc.tensor.matmul(
                                pa[:, 512 + col0:1024],
                                lhsT=kT_sb[64:128, h, kt * 128:(kt + 1) * 128],
                                rhs=qT_sb[64:128, h, s * 512 + col0:(s + 1) * 512],
                                start=True, stop=True, tile_position=(64, 0),
                                skip_group_check=True,
                            )
                            ee = epools[h].tile([128, 1024], f16, tag="e")
                            # exp both half-heads in one ACT op (strided AP
                            # skips the invalid leading columns of each half)
                            nc.scalar.activation(
                                ee.rearrange("p (b c) -> p b c", b=2)[:, :, col0:512],
                                pa.rearrange("p (b c) -> p b c", b=2)[:, :, col0:512],
                                AF.Exp)
                            if kt >= 4 * s:
                                c = col0
                                nc.gpsimd.tensor_mul(ee[:, c:c + 128],
                                                     ee[:, c:c + 128], maskt)
                                nc.gpsimd.tensor_mul(ee[:, 512 + c:512 + c + 128],
                                                     ee[:, 512 + c:512 + c + 128],
                                                     maskt)
                            e_tiles[kt] = ee
                            if kt >= 4 * s:
                                pv_qtile(h, s, kt - 4 * s, e_tiles, oq_s, ss_s)
                            # interleave deferred work
                            it += 1
                            target = -(-total0 * it // niter)  # ceil
                            while pending and emitted < target:
                                pending.pop(0)()
                                emitted += 1
                        if s == NSTRIP - 1 and h == 1:
                            norm_tail(h, s, oq_s, ss_s, tpool=ps_s,
                                      ttag="sc")
                        else:
                            norm_tail(h, s, oq_s, ss_s)

                outproj_last(NSTRIP - 1)

    nc.compile()
    return nc


def _prep_inputs(x, Wq, Wk, Wv, Wo):
    """Build the 8 per-core input maps (host-side shard + fp8 packing)."""
    import ml_dtypes
    F8 = ml_dtypes.float8_e4m3
    f16 = np.float16

    def q8(a):
        return a.astype(F8)

    scale = HD ** -0.5
    # x packs are per batch: [D, 2, S] with var 0 = x8, var 1 = r8
    xpacks = []
    for b in range(B):
        xT = np.ascontiguousarray(x[b].T) * BX     # (D, S) fp32
        x8 = q8(xT)
        r8 = q8(xT - x8.astype(np.float32))
        xp = np.empty((2, D, S), dtype=F8)
        xp[0] = x8
        xp[1] = r8
        xpacks.append(xp)

    def wpack(W, a, pre=1.0):
        # W slice already (rows 256, D); computes pack of (W.T * pre) * a
        WT = np.ascontiguousarray(W.T) * (pre * a)  # (D, 256)
        W8 = q8(WT)
        rW8 = q8(WT - W8.astype(np.float32))
        wp = np.empty((2, D, 256), dtype=F8)
        wp[0] = rW8
        wp[1] = W8
        return wp

    in_maps = []
    for d in range(N_CORES):
        b, p = divmod(d, 4)
        r0 = 256 * p
        in_maps.append({
            "xpack": xpacks[b],
            "wqp": wpack(Wq[r0:r0 + 256, :], AQ, pre=scale),
            "wkp": wpack(Wk[r0:r0 + 256, :], AK),
            "wvp": wpack(Wv[r0:r0 + 256, :], AV),
            "woT": np.ascontiguousarray(Wo[:, r0:r0 + 256].T).astype(f16),
        })
    return in_maps


_CACHED = {}


def _get_program(lam: float):
    # the program depends on inputs only through lam
    key = round(float(lam), 9)
    if key not in _CACHED:
        _CACHED[key] = _build_program(float(lam))
    return _CACHED[key]


def kernel(x, Wq, Wk, Wv, Wo, lq1, lk1, lq2, lk2):
    from concourse.bass_utils import run_bass_kernel_spmd

    x = np.asarray(x, dtype=np.float32)
    Wq = np.asarray(Wq, dtype=np.float32)
    Wk = np.asarray(Wk, dtype=np.float32)
    Wv = np.asarray(Wv, dtype=np.float32)
    Wo = np.asarray(Wo, dtype=np.float32)
    lq1 = np.asarray(lq1, dtype=np.float32)
    lk1 = np.asarray(lk1, dtype=np.float32)
    lq2 = np.asarray(lq2, dtype=np.float32)
    lk2 = np.asarray(lk2, dtype=np.float32)

    lam1 = np.exp(np.sum(lq1 * lk1, dtype=np.float32))
    lam2 = np.exp(np.sum(lq2 * lk2, dtype=np.float32))
    lam = float(lam1 - lam2 + LAMBDA_INIT)

    nc = _get_program(lam)
    in_maps = _prep_inputs(x, Wq, Wk, Wv, Wo)
    res = run_bass_kernel_spmd(nc, in_maps, core_ids=list(range(N_CORES)))

    out = np.empty((B, S, D), dtype=np.float32)
    for b in range(B):
        acc = res.results[4 * b]["outT"].astype(np.float32)
        for p in range(1, 4):
            acc += res.results[4 * b + p]["outT"].astype(np.float32)
        out[b] = acc.T
    return out


# revision 58
# speedup vs baseline: 1.1830x; 1.0295x over previous
"""DiffAttention Trainium2 kernel (v2).

Full inputs in, full output out. Sharding: 8 cores = (batch b in {0,1}) x
(head-pair p in {0..3}); each core handles one batch element and 2 of the 8
heads (= 4 of the 16 q/k half-heads, 2 v heads, 256 of the 1024 o columns).
Out-projection is column-split: each core produces a full (S, D) partial of
o @ Wo.T restricted to its o columns; host sums the 4 partials per batch.

v2 changes over the baseline:
- q/k/v projections run in fp8e4m3 with DoubleRow perf mode (0.5 cyc/row,
  256-deep contraction). Accuracy is restored with a 3-chain residual
  decomposition: with x~=(x8+r8)/b and W~=(W8+rW8)/a (operands pre-scaled
  into fp8's sweet range host-side), x@W = [x8@W8 + (x8@rW8 + r8@W8)]/(ab),
  dropping the O(eps^2) r8@rW8 term. The main chain is 4 DoubleRow matmuls
  pairing kt tiles of x8/W8; the correction is 8 DoubleRow matmuls pairing
  (x8,r8) against (rW8,W8) per kt — together 12 DR matmuls (6 eff. rows)
  versus 8 fp16 matmuls, a 25% PE saving at ~2e-3 overall rel err.
- Projections are emitted per 512-column S-chunk and software-pipelined into
  the attention strip loop (strip s consumes chunk s; chunk s+1's projection
  groups are interleaved between strip s's kt iterations). This overlaps the
  exp-heavy ACT phase with projection PE work and fills PE dependency
  bubbles with independent projection matmuls.
- Evacuation copies use nc.any so the Tile scheduler balances DVE/ACT.
- Input DMAs avoid the ACT queue (exp is the second-busiest engine);
  weight/x DMAs are split fine-grained so the first projection starts ~3us
  in; output DMA is split in half per strip to shorten the kernel tail.

Attention math per head h (half-heads e0=2h, e1=2h+1), per q row:
  u_i = exp(s_i) @ v   (unnormalized), sum_i = exp(s_i) @ 1  (fused: rhs=[v|1])
  o   = u0/sum0 - lam * u1/sum1
  o   = o * rsqrt(mean(o^2)+eps) * (1-lam_init);   out = o @ Wo.T
Scores are computed transposed (keys on partitions, q on free dim) so the
exp'd tiles feed the PV matmul directly as the stationary operand. rsqrt is
Newton-Raphson on the DVE (fast-inverse-sqrt seed), batched per (strip, head),
keeping the ACT engine exp-only. PSUM banks: scores 2x2, u 2x1, and a shared
2x1 pool for projection chunks, o^T transposes and the out-projection.
"""

import math

import numpy as np

B = 2
S = 2048
D = 1024
H = 8
HD = 64  # half-head dim
LAMBDA_INIT = 0.8 - 0.6 * math.exp(-0.3 * 6)
EPS = 1e-5

N_CORES = 8
KT = D // 128       # 8 contraction tiles for projections
ST = S // 128       # 16 sequence tiles
NSTRIP = S // 512   # 4 q strips

BX = 16.0     # x pre-quantization scale
AQ = 1024.0   # Wq.T * hd^-0.5 scale
AK = 128.0    # Wk.T scale
AV = 128.0    # Wv.T scale


def _build_program(lam: float):
    import concourse.bass as bass
    import concourse.tile as tile
    from concourse import bacc, mybir
    from concourse.masks import make_identity

    f16 = mybir.dt.float16
    f32 = mybir.dt.float32
    f8 = mybir.dt.float8e4
    u32 = mybir.dt.uint32
    AF = mybir.ActivationFunctionType
    OP = mybir.AluOpType
    DR = mybir.MatmulPerfMode.DoubleRow

    nc = bacc.Bacc("TRN2", target_bir_lowering=False, debug=False,
                   num_devices=N_CORES)

    # packed fp8 inputs: var index 0 = x8 (scaled main), 1 = r8 (residual)
    # weights:           var index 0 = rW8 (residual), 1 = W8 (scaled main)
    xpack = nc.dram_tensor("xpack", (2, D, S), f8, kind="ExternalInput").ap()
    wqp = nc.dram_tensor("wqp", (2, D, 256), f8, kind="ExternalInput").ap()
    wkp = nc.dram_tensor("wkp", (2, D, 256), f8, kind="ExternalInput").ap()
    wvp = nc.dram_tensor("wvp", (2, D, 256), f8, kind="ExternalInput").ap()
    woT = nc.dram_tensor("woT", (256, D), f16, kind="ExternalInput").ap()
    outT = nc.dram_tensor("outT", (D, S), f16, kind="ExternalOutput").ap()

    with tile.TileContext(nc) as tc:
        with (
            tc.tile_pool(name="const", bufs=1) as cpool,
            tc.tile_pool(name="persist", bufs=1) as pp,
        ):
            ident = cpool.tile([128, 128], f16, tag="ident")
            make_identity(nc, ident)
            # mask[p, f] = 1 if p <= f else 0 (keys on partitions, q on free)
            maskt = cpool.tile([128, 128], f16, tag="maskt")
            nc.gpsimd.memset(maskt, 1.0)
            nc.gpsimd.affine_select(
                out=maskt, in_=maskt, compare_op=OP.is_ge, fill=0.0,
                base=0, pattern=[[1, 128]], channel_multiplier=-1,
            )

            # constants for Newton-Raphson rsqrt (fast-inverse-sqrt seed)
            magic_c = cpool.tile([128, 8], u32, tag="magic_c")
            nc.gpsimd.memset(magic_c, 0x5F3759DF)
            one_u = cpool.tile([128, 8], u32, tag="one_u")
            nc.gpsimd.memset(one_u, 1)

            wo_sb = pp.tile([128, 2, D], f16, tag="wo_sb")
            qT_sb = pp.tile([128, 2, S], f16, tag="qT_sb")
            kT_sb = pp.tile([128, 2, S], f16, tag="kT_sb")
            # v with a ones column appended per head: [v_h0 | 1 | v_h1 | 1]
            v_sb = pp.tile([128, ST, 258], f16, tag="v_sb")
            nc.vector.memset(v_sb[:, :, 128:129], 1.0)
            nc.vector.memset(v_sb[:, :, 257:258], 1.0)
            oT_sb = pp.tile([128, 2, S], f16, tag="oT_sb")

            # fp8 packed inputs stay resident for the whole kernel
            xp_sb = pp.tile([128, 2, KT, S], f8, tag="xp_sb")
            wq_sb = pp.tile([128, 2, KT, 256], f8, tag="wq_sb")
            wk_sb = pp.tile([128, 2, KT, 256], f8, tag="wk_sb")
            wv_sb = pp.tile([128, 2, KT, 256], f8, tag="wv_sb")

            xp_r = xpack.rearrange("v (kt p) s -> p v kt s", p=128)
            wq_r = wqp.rearrange("v (kt p) m -> p v kt m", p=128)
            wk_r = wkp.rearrange("v (kt p) m -> p v kt m", p=128)
            wv_r = wvp.rearrange("v (kt p) m -> p v kt m", p=128)

            # weight DMAs on Pool queue (ACT must stay exp-only), x chunks on
            # SP, main (v=0) halves before residual (v=1) so the first
            # projection matmuls can start early.
            nc.gpsimd.dma_start(wq_sb[:, 1, 0:2, :], wq_r[:, 1, 0:2, :])
            nc.gpsimd.dma_start(wq_sb[:, 1, 2:4, :], wq_r[:, 1, 2:4, :])
            nc.gpsimd.dma_start(wq_sb[:, 1, 4:8, :], wq_r[:, 1, 4:8, :])
            nc.gpsimd.dma_start(wk_sb[:, 1, :, :], wk_r[:, 1, :, :])
            nc.gpsimd.dma_start(wq_sb[:, 0, :, :], wq_r[:, 0, :, :])
            nc.gpsimd.dma_start(wk_sb[:, 0, :, :], wk_r[:, 0, :, :])
            nc.sync.dma_start(xp_sb[:, 0, 0:4, 0:512], xp_r[:, 0, 0:4, 0:512])
            nc.sync.dma_start(xp_sb[:, 0, 4:8, 0:512], xp_r[:, 0, 4:8, 0:512])
            nc.sync.dma_start(xp_sb[:, 1, :, 0:512], xp_r[:, 1, :, 0:512])
            for v in (1, 0):
                nc.gpsimd.dma_start(wv_sb[:, v, :, :], wv_r[:, v, :, :])
            for c in range(1, 4):
                sl = slice(c * 512, (c + 1) * 512)
                nc.sync.dma_start(xp_sb[:, 0, :, sl], xp_r[:, 0, :, sl])
                nc.sync.dma_start(xp_sb[:, 1, :, sl], xp_r[:, 1, :, sl])
            nc.gpsimd.dma_start(
                wo_sb[:, :, :],
                woT.rearrange("(kt p) n -> p kt n", p=128)[:, :, :])

            with (
                tc.tile_pool(name="e0p", bufs=16) as e0pool,
                tc.tile_pool(name="e1p", bufs=16) as e1pool,
                tc.tile_pool(name="ps_s", bufs=2, space="PSUM") as ps_s,
                tc.tile_pool(name="ps_u", bufs=2, space="PSUM") as ps_u,
                tc.tile_pool(name="po", bufs=2, space="PSUM") as po,
                tc.tile_pool(name="nrm", bufs=8) as nrm,
                tc.tile_pool(name="nrm_big", bufs=3) as nrm_big,
                tc.tile_pool(name="osb", bufs=3) as osb,
                tc.tile_pool(name="out_sb", bufs=2) as out_pool,
                tc.tile_pool(name="pre3", bufs=12) as pre_pool,
            ):
                epools = {0: e0pool, 1: e1pool}
                outT_r = outT.rearrange("(mt p) s -> p mt s", p=128)

                # ---------------- fp8 projection groups ----------------
                def proj_qk_main(w_sb, mt, c):
                    ps = po.tile([128, 512], f32, tag="po")
                    msl = slice(mt * 128, (mt + 1) * 128)
                    csl = slice(c * 512, (c + 1) * 512)
                    for j in range(4):
                        nc.tensor.matmul(
                            ps[:],
                            lhsT=w_sb[:, 1, 2 * j:2 * j + 2, msl],
                            rhs=xp_sb[:, 0, 2 * j:2 * j + 2, csl],
                            start=(j == 0), stop=False, perf_mode=DR,
                        )
                    return ps

                def proj_qk_corr(ps, w_sb, dst_sb, mt, c, inv_ab):
                    msl = slice(mt * 128, (mt + 1) * 128)
                    csl = slice(c * 512, (c + 1) * 512)
                    for kt in range(KT):
                        nc.tensor.matmul(
                            ps[:],
                            lhsT=w_sb[:, :, kt, msl],
                            rhs=xp_sb[:, :, kt, csl],
                            start=False, stop=(kt == KT - 1), perf_mode=DR,
                        )
                    nc.vector.tensor_scalar(dst_sb[:, mt, csl], ps[:], inv_ab,
                                            None, OP.mult)

                def emit_proj_qk(w_sb, dst_sb, mt, c, inv_ab):
                    ps = proj_qk_main(w_sb, mt, c)
                    proj_qk_corr(ps, w_sb, dst_sb, mt, c, inv_ab)

                def emit_proj_v(st, inv_ab):
                    ps = po.tile([128, 512], f32, tag="po")
                    ssl = slice(st * 128, (st + 1) * 128)
                    for j in range(4):
                        nc.tensor.matmul(
                            ps[:, 0:256],
                            lhsT=xp_sb[:, 0, 2 * j:2 * j + 2, ssl],
                            rhs=wv_sb[:, 1, 2 * j:2 * j + 2, :],
                            start=(j == 0), stop=False, perf_mode=DR,
                        )
                    for kt in range(KT):
                        nc.tensor.matmul(
                            ps[:, 0:256],
                            lhsT=xp_sb[:, :, kt, ssl],
                            rhs=wv_sb[:, :, kt, :],
                            start=False, stop=(kt == KT - 1), perf_mode=DR,
                        )
                    nc.vector.tensor_scalar(v_sb[:, st, 0:128], ps[:, 0:128],
                                            inv_ab, None, OP.mult)
                    nc.vector.tensor_scalar(v_sb[:, st, 129:257],
                                            ps[:, 128:256], inv_ab, None,
                                            OP.mult)

                def proj_q_groups(c):
                    return [lambda mt=mt: emit_proj_qk(
                        wq_sb, qT_sb, mt, c, 1.0 / (AQ * BX))
                        for mt in range(2)]

                def proj_kv_groups(c):
                    gs = [lambda mt=mt: emit_proj_qk(
                        wk_sb, kT_sb, mt, c, 1.0 / (AK * BX))
                        for mt in range(2)]
                    for st in range(4 * c, 4 * c + 4):
                        gs.append(lambda st=st: emit_proj_v(st, 1.0 / (AV * BX)))
                    return gs

                def proj_chunk_groups(c):
                    gs = []
                    qg = proj_q_groups(c)
                    kg = proj_kv_groups(c)
                    gs += [qg[0], kg[0], qg[1], kg[1]] + kg[2:]
                    return gs

                # ---------------- attention ----------------
                def pv_qtile(h, s, i, e_tiles, oq_s, ss_s):
                    qt = 4 * s + i
                    up = ps_u.tile([128, 258], f32, tag="u")
                    for kt in range(qt + 1):
                        c = i * 128
                        vh = v_sb[:, kt, 129 * h:129 * h + 129]
                        nc.tensor.matmul(
                            up[:, 0:129],
                            lhsT=e_tiles[kt][:, c:c + 128],
                            rhs=vh,
                            start=(kt == 0), stop=(kt == qt),
                        )
                        nc.tensor.matmul(
                            up[:, 129:258],
                            lhsT=e_tiles[kt][:, 512 + c:512 + c + 128],
                            rhs=vh,
                            start=False, stop=(kt == qt),
                            skip_group_check=True,
                        )
                    # normalized diff: oq = u0/s0 - lam*u1/s1 (per-partition)
                    inv0 = nrm.tile([128, 1], f32, tag="inv0")
                    nc.vector.reciprocal(inv0, up[:, 128:129])
                    inv1 = nrm.tile([128, 1], f32, tag="inv1")
                    nc.vector.reciprocal(inv1, up[:, 257:258])
                    t1 = nrm.tile([128, 128], f32, tag="t1")
                    nc.vector.tensor_scalar(t1, up[:, 129:257], inv1, lam,
                                            OP.mult, OP.mult)
                    oq = oq_s[:, i, :]
                    nc.vector.scalar_tensor_tensor(
                        oq, up[:, 0:128], inv0, t1, OP.mult, OP.subtract)
                    sq = nrm.tile([128, 128], f32, tag="sq")
                    nc.vector.scalar_tensor_tensor(
                        sq, oq, 1.0, oq, OP.bypass, OP.mult,
                        accum_out=ss_s[:, i:i + 1])

                def norm_tail(h, s, oq_s, ss_s, tpool=None, ttag="po"):
                    """Newton rsqrt over the strip's 4 q-tiles, then scale,
                    transpose and evacuate o^T."""
                    if tpool is None:
                        tpool = po
                    ms = nrm.tile([128, 4], f32, tag="ms")
                    il2 = 1.0 / ((1.0 - LAMBDA_INIT) ** 2)
                    nc.vector.tensor_scalar(ms, ss_s, il2 / 128.0, EPS * il2,
                                            OP.mult, OP.add)
                    y0 = nrm.tile([128, 4], u32, tag="y0")
                    nc.vector.tensor_tensor(y0, ms.bitcast(u32),
                                            one_u[:, 0:4],
                                            OP.logical_shift_right)
                    nc.vector.tensor_tensor(y0, magic_c[:, 0:4], y0,
                                            OP.subtract)
                    yf = y0.bitcast(f32)
                    t2 = nrm.tile([128, 4], f32, tag="t2")
                    r_all = nrm.tile([128, 4], f32, tag="r_all")
                    nc.vector.tensor_mul(t2, yf, yf)
                    nc.vector.tensor_mul(t2, t2, ms)
                    nc.vector.tensor_scalar(t2, t2, -0.5, 1.5, OP.mult, OP.add)
                    nc.vector.tensor_mul(r_all, yf, t2)
                    nc.vector.tensor_mul(t2, r_all, r_all)
                    nc.vector.tensor_mul(t2, t2, ms)
                    nc.vector.tensor_scalar(t2, t2, -0.5, 1.5, OP.mult, OP.add)
                    nc.vector.tensor_mul(r_all, r_all, t2)
                    for i in range(4):
                        qt = 4 * s + i
                        on = osb.tile([128, 128], f16, tag="on")
                        nc.vector.tensor_scalar(on, oq_s[:, i, :],
                                                r_all[:, i:i + 1], None,
                                                OP.mult)
                        pt = tpool.tile([128, 512], f16, tag=ttag)
                        nc.tensor.transpose(pt[:, 0:128], on, ident)
                        nc.vector.tensor_copy(
                            oT_sb[:, h, qt * 128:(qt + 1) * 128], pt[:, 0:128])

                def outproj_groups(s):
                    """8 filler groups (one per mt) computing strip s's
                    out-projection; interleaved into strip s+1."""
                    state = {}

                    def grp(mt):
                        if mt == 0:
                            ot = out_pool.tile([128, 8, 512], f16, tag="ot")
                            state["ot"] = ot
                        ot = state["ot"]
                        ps = po.tile([128, 512], f32, tag="po")
                        for kt in range(2):
                            nc.tensor.matmul(
                                ps[:],
                                lhsT=wo_sb[:, kt, mt * 128:(mt + 1) * 128],
                                rhs=oT_sb[:, kt, s * 512:(s + 1) * 512],
                                start=(kt == 0), stop=(kt == 1),
                            )
                        nc.vector.tensor_copy(ot[:, mt, :], ps[:])
                        if mt == 3:
                            nc.sync.dma_start(
                                outT_r[:, 0:4, s * 512:(s + 1) * 512],
                                ot[:, 0:4, :])
                        elif mt == 7:
                            nc.sync.dma_start(
                                outT_r[:, 4:8, s * 512:(s + 1) * 512],
                                ot[:, 4:8, :])

                    return [lambda mt=mt: grp(mt) for mt in range(8)]

                def outproj_last(s):
                    """Final strip: per-128-column sub-tile matmuls so 3/4 of
                    the PE work overlaps the final norm_tail; grouped evacs
                    alternate DVE/ACT; DMA per 2 mt."""
                    ot = out_pool.tile([128, 8, 512], f16, tag="ot")
                    for mt in range(8):
                        # borrow the idle scores/u psum slots: 6 groups in
                        # flight instead of 2
                        if mt % 3 == 0:
                            ps = po.tile([128, 512], f32, tag="po")
                        elif mt % 3 == 1:
                            ps = ps_s.tile([128, 512], f32, tag="sc")
                        else:
                            ps = ps_u.tile([128, 512], f32, tag="u")
                        for sub in range(4):
                            for kt in range(2):
                                nc.tensor.matmul(
                                    ps[:, sub * 128:(sub + 1) * 128],
                                    lhsT=wo_sb[:, kt, mt * 128:(mt + 1) * 128],
                                    rhs=oT_sb[:, kt,
                                              s * 512 + sub * 128:
                                              s * 512 + (sub + 1) * 128],
                                    start=(kt == 0), stop=(kt == 1),
                                    skip_group_check=(sub > 0),
                                )
                        if mt % 2 == 1:
                            nc.scalar.copy(ot[:, mt, :], ps[:])
                            q = nc.sync if mt % 4 == 1 else nc.gpsimd
                            q.dma_start(
                                outT_r[:, mt - 1:mt + 1,
                                       s * 512:(s + 1) * 512],
                                ot[:, mt - 1:mt + 1, :])
                        else:
                            nc.vector.tensor_copy(ot[:, mt, :], ps[:])

                pre_tiles = {}

                def early_sc_group(h, kt, strip=3, pool=None):
                    sl = strip * 512
                    pa = ps_s.tile([128, 1024], f32, tag="sc")
                    nc.tensor.matmul(
                        pa[:, 0:512],
                        lhsT=kT_sb[0:64, h, kt * 128:(kt + 1) * 128],
                        rhs=qT_sb[0:64, h, sl:sl + 512],
                        start=True, stop=True, tile_position=(0, 0),
                    )
                    nc.tensor.matmul(
                        pa[:, 512:1024],
                        lhsT=kT_sb[64:128, h, kt * 128:(kt + 1) * 128],
                        rhs=qT_sb[64:128, h, sl:sl + 512],
                        start=True, stop=True, tile_position=(64, 0),
                        skip_group_check=True,
                    )
                    if pool is None:
                        ee = pre_pool.tile([128, 1024], f16, tag="pe3")
                    else:
                        ee = pool.tile([128, 1024], f16, tag="e")
                    nc.scalar.activation(ee, pa, AF.Exp)
                    pre_tiles[(strip, h, kt)] = ee

                NPRE = 6  # strip-3 kt tiles precomputed per head

                # chunk 0 projections up front; the first two groups'
                # main chains run before any correction chain so the PE
                # isn't blocked in-order on the residual (v=1) DMAs
                ps_q0 = proj_qk_main(wq_sb, 0, 0)
                ps_k0 = proj_qk_main(wk_sb, 0, 0)
                proj_qk_corr(ps_q0, wq_sb, qT_sb, 0, 0, 1.0 / (AQ * BX))
                proj_qk_corr(ps_k0, wk_sb, kT_sb, 0, 0, 1.0 / (AK * BX))
                emit_proj_qk(wq_sb, qT_sb, 1, 0, 1.0 / (AQ * BX))
                emit_proj_qk(wk_sb, kT_sb, 1, 0, 1.0 / (AK * BX))
                for st in range(4):
                    emit_proj_v(st, 1.0 / (AV * BX))

                for s in range(NSTRIP):
                    pending = []
                    if s == 0:
                        pending += proj_chunk_groups(1)
                        pending += proj_q_groups(2) + proj_q_groups(3)
                        pending += [lambda h=h, kt=kt: early_sc_group(
                                        h, kt, strip=1, pool=epools[h])
                                    for h in range(2) for kt in range(4)]
                    elif s == 1:
                        early = [lambda h=h, kt=kt: early_sc_group(h, kt)
                                 for h in range(2) for kt in range(NPRE)]
                        early += [lambda h=h, kt=kt: early_sc_group(
                                      h, kt, strip=2, pool=epools[h])
                                  for h in range(2) for kt in range(2)]
                        rest = outproj_groups(0) + proj_kv_groups(2)
                        pending = [g for pair in zip(early, rest)
                                   for g in pair]
                        pending += early[len(rest):] + rest[len(early):]
                    elif s == 2:
                        pending += outproj_groups(1) + proj_kv_groups(3)
                    else:
                        pending += outproj_groups(2)
                    niter = 2 * 4 * (s + 1)
                    total0 = len(pending)
                    emitted = 0
                    it = 0
                    for h in range(2):
                        e_tiles = {}
                        oq_s = nrm_big.tile([128, 4, 128], f32, tag="oq_s")
                        ss_s = nrm_big.tile([128, 4], f32, tag="ss_s")
                        for kt in range(4 * (s + 1)):
                            npre_s = (NPRE if s == 3 else
                                      4 if s == 1 else
                                      2 if s == 2 else 0)
                            if kt < npre_s:
                                e_tiles[kt] = pre_tiles[(s, h, kt)]
                                it += 1
                                target = -(-total0 * it // niter)  # ceil
                                while pending and emitted < target:
                                    pending.pop(0)()
                                    emitted += 1
                                continue
                            col0 = max(0, (kt - 4 * s) * 128)
                            pa = ps_s.tile([128, 1024], f32, tag="sc")
                            nc.tensor.matmul(
                                pa[:, col0:512],
                                lhsT=kT_sb[0:64, h, kt * 128:(kt + 1) * 128],
                                rhs=qT_sb[0:64, h, s * 512 + col0:(s + 1) * 512],
                                start=True, stop=True, tile_position=(0, 0),
                            )
                            nc.tensor.matmul(
                                pa[:, 512 + col0:1024],
                                lhsT=kT_sb[64:128, h, kt * 128:(kt + 1) * 128],
                                rhs=qT_sb[64:128, h, s * 512 + col0:(s + 1) * 512],
                                start=True, stop=True, tile_position=(64, 0),
                                skip_group_check=True,
                            )
                            ee = epools[h].tile([128, 1024], f16, tag="e")
                            # exp both half-heads in one ACT op (strided AP
                            # skips the invalid leading columns of each half)
                            nc.scalar.activation(
                                ee.rearrange("p (b c) -> p b c", b=2)[:, :, col0:512],
                                pa.rearrange("p (b c) -> p b c", b=2)[:, :, col0:512],
                                AF.Exp)
                            if kt >= 4 * s:
                                c = col0
                                nc.gpsimd.tensor_mul(ee[:, c:c + 128],
                                                     ee[:, c:c + 128], maskt)
                                nc.gpsimd.tensor_mul(ee[:, 512 + c:512 + c + 128],
                                                     ee[:, 512 + c:512 + c + 128],
                                                     maskt)
                            e_tiles[kt] = ee
                            if kt >= 4 * s:
                                pv_qtile(h, s, kt - 4 * s, e_tiles, oq_s, ss_s)
                            # interleave deferred work
                            it += 1
                            target = -(-total0 * it // niter)  # ceil
                            while pending and emitted < target:
                                pending.pop(0)()
                                emitted += 1
                        if s == NSTRIP - 1 and h == 1:
                            norm_tail(h, s, oq_s, ss_s, tpool=ps_s,
                                      ttag="sc")
                        else:
                            norm_tail(h, s, oq_s, ss_s)

                outproj_last(NSTRIP - 1)

    nc.compile()
    return nc


def _prep_inputs(x, Wq, Wk, Wv, Wo):
    """Build the 8 per-core input maps (host-side shard + fp8 packing)."""
    import ml_dtypes
    F8 = ml_dtypes.float8_e4m3
    f16 = np.float16

    def q8(a):
        return a.astype(F8)

    scale = HD ** -0.5
    # x packs are per batch: [D, 2, S] with var 0 = x8, var 1 = r8
    xpacks = []
    for b in range(B):
        xT = np.ascontiguousarray(x[b].T) * BX     # (D, S) fp32
        x8 = q8(xT)
        r8 = q8(xT - x8.astype(np.float32))
        xp = np.empty((2, D, S), dtype=F8)
        xp[0] = x8
        xp[1] = r8
        xpacks.append(xp)

    def wpack(W, a, pre=1.0):
        # W slice already (rows 256, D); computes pack of (W.T * pre) * a
        WT = np.ascontiguousarray(W.T) * (pre * a)  # (D, 256)
        W8 = q8(WT)
        rW8 = q8(WT - W8.astype(np.float32))
        wp = np.empty((2, D, 256), dtype=F8)
        wp[0] = rW8
        wp[1] = W8
        return wp

    in_maps = []
    for d in range(N_CORES):
        b, p = divmod(d, 4)
        r0 = 256 * p
        in_maps.append({
            "xpack": xpacks[b],
            "wqp": wpack(Wq[r0:r0 + 256, :], AQ, pre=scale),
            "wkp": wpack(Wk[r0:r0 + 256, :], AK),
            "wvp": wpack(Wv[r0:r0 + 256, :], AV),
            "woT": np.ascontiguousarray(Wo[:, r0:r0 + 256].T).astype(f16),
        })
    return in_maps


_CACHED = {}


def _get_program(lam: float):
    # the program depends on inputs only through lam
    key = round(float(lam), 9)
    if key not in _CACHED:
        _CACHED[key] = _build_program(float(lam))
    return _CACHED[key]


def kernel(x, Wq, Wk, Wv, Wo, lq1, lk1, lq2, lk2):
    from concourse.bass_utils import run_bass_kernel_spmd

    x = np.asarray(x, dtype=np.float32)
    Wq = np.asarray(Wq, dtype=np.float32)
    Wk = np.asarray(Wk, dtype=np.float32)
    Wv = np.asarray(Wv, dtype=np.float32)
    Wo = np.asarray(Wo, dtype=np.float32)
    lq1 = np.asarray(lq1, dtype=np.float32)
    lk1 = np.asarray(lk1, dtype=np.float32)
    lq2 = np.asarray(lq2, dtype=np.float32)
    lk2 = np.asarray(lk2, dtype=np.float32)

    lam1 = np.exp(np.sum(lq1 * lk1, dtype=np.float32))
    lam2 = np.exp(np.sum(lq2 * lk2, dtype=np.float32))
    lam = float(lam1 - lam2 + LAMBDA_INIT)

    nc = _get_program(lam)
    in_maps = _prep_inputs(x, Wq, Wk, Wv, Wo)
    res = run_bass_kernel_spmd(nc, in_maps, core_ids=list(range(N_CORES)))

    out = np.empty((B, S, D), dtype=np.float32)
    for b in range(B):
        acc = res.results[4 * b]["outT"].astype(np.float32)
        for p in range(1, 4):
            acc += res.results[4 * b + p]["outT"].astype(np.float32)
        out[b] = acc.T
    return out


# revision 69
# speedup vs baseline: 1.1866x; 1.0030x over previous
"""DiffAttention Trainium2 kernel (v2).

Full inputs in, full output out. Sharding: 8 cores = (batch b in {0,1}) x
(head-pair p in {0..3}); each core handles one batch element and 2 of the 8
heads (= 4 of the 16 q/k half-heads, 2 v heads, 256 of the 1024 o columns).
Out-projection is column-split: each core produces a full (S, D) partial of
o @ Wo.T restricted to its o columns; host sums the 4 partials per batch.

Key optimizations over the fp16 baseline (150.5us -> 126.9us CoreSim):

1. fp8 DoubleRow projections (q/k/v). DoubleRow contracts 2x128 partitions
   per matmul at 0.5 cycles/row. Accuracy is restored with a 3-chain
   residual decomposition: host pre-scales x and W into fp8e4m3's sweet
   range (avoiding subnormals), splits each into a main fp8 value plus an
   fp8 residual (x~=(x8+r8)/b, W~=(W8+rW8)/a), and the kernel computes
   x@W*ab = x8@W8 (4 DoubleRow matmuls pairing kt tiles) + [x8@rW8+r8@W8]
   (8 DoubleRow matmuls pairing the var dim), dropping the O(eps^2)
   r8@rW8 term. 12 DR matmuls = 6 effective fp16-rows vs 8 for fp16: 25%
   PE saving on projections at ~2e-3 overall rel err (vs 7.4e-4 all-fp16).
   The 1/(ab) descale is folded into the PSUM evacuation multiply.

2. Software-pipelined projections. Projections are emitted per 512-column
   S-chunk; strip s of the attention consumes chunk s, and chunk s+1's
   projection groups (plus the previous strip's deferred out-projection)
   are interleaved between strip s's kt iterations as elastic PE filler.

3. Early score/exp precompute. The causal structure makes late strips
   ACT(exp)-bound and early strips PE-bound. Score+exp tiles for the first
   kt tiles of strips 1/2/3 (4/2/6 per head) are computed during earlier
   strips where the ACT engine idles, shifting ~14us of exp off the
   ACT-bound tail region. q projections for all chunks run during strip 0
   to enable this.

4. Tail/start scheduling: weight/x DMAs split so the first DoubleRow
   matmul starts ~2.9us in (main-chain operands land before residuals);
   the final strip's out-projection runs per-128-column sub-tiles gated on
   individual norm_tail outputs, rotates PSUM across the then-idle
   score/u pools (6 groups in flight), alternates DVE/ACT evacuation and
   splits output DMAs across the SP and Pool queues; the final RMSNorm is
   split 3+1 so most of its DVE chain overlaps the last PV burst.

Attention math per head h (half-heads e0=2h, e1=2h+1), per q row:
  u_i = exp(s_i) @ v   (unnormalized), sum_i = exp(s_i) @ 1  (fused: rhs=[v|1])
  o   = u0/sum0 - lam * u1/sum1
  o   = o * rsqrt(mean(o^2)+eps) * (1-lam_init);   out = o @ Wo.T
Scores are computed transposed (keys on partitions, q on free dim) so the
exp'd tiles feed the PV matmul directly as the stationary operand. rsqrt is
Newton-Raphson on the DVE (fast-inverse-sqrt seed) — ACT Ln+Exp would
ping-pong activation tables (1.3us per reload). PSUM banks: scores 2x2,
u 2x1, and a shared 2x1 pool for projection chunks, o^T transposes and the
out-projection.
"""

import math

import numpy as np

B = 2
S = 2048
D = 1024
H = 8
HD = 64  # half-head dim
LAMBDA_INIT = 0.8 - 0.6 * math.exp(-0.3 * 6)
EPS = 1e-5

N_CORES = 8
KT = D // 128       # 8 contraction tiles for projections
ST = S // 128       # 16 sequence tiles
NSTRIP = S // 512   # 4 q strips

BX = 16.0     # x pre-quantization scale
AQ = 1024.0   # Wq.T * hd^-0.5 scale
AK = 128.0    # Wk.T scale
AV = 128.0    # Wv.T scale


def _build_program(lam: float):
    import concourse.bass as bass
    import concourse.tile as tile
    from concourse import bacc, mybir
    from concourse.masks import make_identity

    f16 = mybir.dt.float16
    f32 = mybir.dt.float32
    f8 = mybir.dt.float8e4
    u32 = mybir.dt.uint32
    AF = mybir.ActivationFunctionType
    OP = mybir.AluOpType
    DR = mybir.MatmulPerfMode.DoubleRow

    nc = bacc.Bacc("TRN2", target_bir_lowering=False, debug=False,
                   num_devices=N_CORES)

    # packed fp8 inputs: var index 0 = x8 (scaled main), 1 = r8 (residual)
    # weights:           var index 0 = rW8 (residual), 1 = W8 (scaled main)
    xpack = nc.dram_tensor("xpack", (2, D, S), f8, kind="ExternalInput").ap()
    wqp = nc.dram_tensor("wqp", (2, D, 256), f8, kind="ExternalInput").ap()
    wkp = nc.dram_tensor("wkp", (2, D, 256), f8, kind="ExternalInput").ap()
    wvp = nc.dram_tensor("wvp", (2, D, 256), f8, kind="ExternalInput").ap()
    woT = nc.dram_tensor("woT", (256, D), f16, kind="ExternalInput").ap()
    outT = nc.dram_tensor("outT", (D, S), f16, kind="ExternalOutput").ap()

    with tile.TileContext(nc) as tc:
        with (
            tc.tile_pool(name="const", bufs=1) as cpool,
            tc.tile_pool(name="persist", bufs=1) as pp,
        ):
            ident = cpool.tile([128, 128], f16, tag="ident")
            make_identity(nc, ident)
            # mask[p, f] = 1 if p <= f else 0 (keys on partitions, q on free)
            maskt = cpool.tile([128, 128], f16, tag="maskt")
            nc.gpsimd.memset(maskt, 1.0)
            nc.gpsimd.affine_select(
                out=maskt, in_=maskt, compare_op=OP.is_ge, fill=0.0,
                base=0, pattern=[[1, 128]], channel_multiplier=-1,
            )

            # constants for Newton-Raphson rsqrt (fast-inverse-sqrt seed)
            magic_c = cpool.tile([128, 8], u32, tag="magic_c")
            nc.gpsimd.memset(magic_c, 0x5F3759DF)
            one_u = cpool.tile([128, 8], u32, tag="one_u")
            nc.gpsimd.memset(one_u, 1)

            wo_sb = pp.tile([128, 2, D], f16, tag="wo_sb")
            qT_sb = pp.tile([128, 2, S], f16, tag="qT_sb")
            kT_sb = pp.tile([128, 2, S], f16, tag="kT_sb")
            # v with a ones column appended per head: [v_h0 | 1 | v_h1 | 1]
            v_sb = pp.tile([128, ST, 258], f16, tag="v_sb")
            nc.vector.memset(v_sb[:, :, 128:129], 1.0)
            nc.vector.memset(v_sb[:, :, 257:258], 1.0)
            oT_sb = pp.tile([128, 2, S], f16, tag="oT_sb")

            # fp8 packed inputs stay resident for the whole kernel
            xp_sb = pp.tile([128, 2, KT, S], f8, tag="xp_sb")
            wq_sb = pp.tile([128, 2, KT, 256], f8, tag="wq_sb")
            wk_sb = pp.tile([128, 2, KT, 256], f8, tag="wk_sb")
            wv_sb = pp.tile([128, 2, KT, 256], f8, tag="wv_sb")

            xp_r = xpack.rearrange("v (kt p) s -> p v kt s", p=128)
            wq_r = wqp.rearrange("v (kt p) m -> p v kt m", p=128)
            wk_r = wkp.rearrange("v (kt p) m -> p v kt m", p=128)
            wv_r = wvp.rearrange("v (kt p) m -> p v kt m", p=128)

            # weight DMAs on Pool queue (ACT must stay exp-only), x chunks on
            # SP, main (v=0) halves before residual (v=1) so the first
            # projection matmuls can start early.
            nc.gpsimd.dma_start(wq_sb[:, 1, 0:2, :], wq_r[:, 1, 0:2, :])
            nc.gpsimd.dma_start(wq_sb[:, 1, 2:4, :], wq_r[:, 1, 2:4, :])
            nc.gpsimd.dma_start(wq_sb[:, 1, 4:8, :], wq_r[:, 1, 4:8, :])
            nc.gpsimd.dma_start(wk_sb[:, 1, :, :], wk_r[:, 1, :, :])
            nc.gpsimd.dma_start(wq_sb[:, 0, :, :], wq_r[:, 0, :, :])
            nc.gpsimd.dma_start(wk_sb[:, 0, :, :], wk_r[:, 0, :, :])
            nc.sync.dma_start(xp_sb[:, 0, 0:4, 0:512], xp_r[:, 0, 0:4, 0:512])
            nc.sync.dma_start(xp_sb[:, 0, 4:8, 0:512], xp_r[:, 0, 4:8, 0:512])
            nc.sync.dma_start(xp_sb[:, 1, :, 0:512], xp_r[:, 1, :, 0:512])
            for v in (1, 0):
                nc.gpsimd.dma_start(wv_sb[:, v, :, :], wv_r[:, v, :, :])
            for c in range(1, 4):
                sl = slice(c * 512, (c + 1) * 512)
                nc.sync.dma_start(xp_sb[:, 0, :, sl], xp_r[:, 0, :, sl])
                nc.sync.dma_start(xp_sb[:, 1, :, sl], xp_r[:, 1, :, sl])
            nc.gpsimd.dma_start(
                wo_sb[:, :, :],
                woT.rearrange("(kt p) n -> p kt n", p=128)[:, :, :])

            with (
                tc.tile_pool(name="e0p", bufs=16) as e0pool,
                tc.tile_pool(name="e1p", bufs=16) as e1pool,
                tc.tile_pool(name="ps_s", bufs=2, space="PSUM") as ps_s,
                tc.tile_pool(name="ps_u", bufs=2, space="PSUM") as ps_u,
                tc.tile_pool(name="po", bufs=2, space="PSUM") as po,
                tc.tile_pool(name="nrm", bufs=8) as nrm,
                tc.tile_pool(name="nrm_big", bufs=3) as nrm_big,
                tc.tile_pool(name="osb", bufs=3) as osb,
                tc.tile_pool(name="out_sb", bufs=2) as out_pool,
                tc.tile_pool(name="pre3", bufs=12) as pre_pool,
            ):
                epools = {0: e0pool, 1: e1pool}
                outT_r = outT.rearrange("(mt p) s -> p mt s", p=128)

                # ---------------- fp8 projection groups ----------------
                def proj_qk_main(w_sb, mt, c):
                    ps = po.tile([128, 512], f32, tag="po")
                    msl = slice(mt * 128, (mt + 1) * 128)
                    csl = slice(c * 512, (c + 1) * 512)
                    for j in range(4):
                        nc.tensor.matmul(
                            ps[:],
                            lhsT=w_sb[:, 1, 2 * j:2 * j + 2, msl],
                            rhs=xp_sb[:, 0, 2 * j:2 * j + 2, csl],
                            start=(j == 0), stop=False, perf_mode=DR,
                        )
                    return ps

                def proj_qk_corr(ps, w_sb, dst_sb, mt, c, inv_ab):
                    msl = slice(mt * 128, (mt + 1) * 128)
                    csl = slice(c * 512, (c + 1) * 512)
                    for kt in range(KT):
                        nc.tensor.matmul(
                            ps[:],
                            lhsT=w_sb[:, :, kt, msl],
                            rhs=xp_sb[:, :, kt, csl],
                            start=False, stop=(kt == KT - 1), perf_mode=DR,
                        )
                    nc.vector.tensor_scalar(dst_sb[:, mt, csl], ps[:], inv_ab,
                                            None, OP.mult)

                def emit_proj_qk(w_sb, dst_sb, mt, c, inv_ab):
                    ps = proj_qk_main(w_sb, mt, c)
                    proj_qk_corr(ps, w_sb, dst_sb, mt, c, inv_ab)

                def emit_proj_v(st, inv_ab):
                    ps = po.tile([128, 512], f32, tag="po")
                    ssl = slice(st * 128, (st + 1) * 128)
                    for j in range(4):
                        nc.tensor.matmul(
                            ps[:, 0:256],
                            lhsT=xp_sb[:, 0, 2 * j:2 * j + 2, ssl],
                            rhs=wv_sb[:, 1, 2 * j:2 * j + 2, :],
                            start=(j == 0), stop=False, perf_mode=DR,
                        )
                    for kt in range(KT):
                        nc.tensor.matmul(
                            ps[:, 0:256],
                            lhsT=xp_sb[:, :, kt, ssl],
                            rhs=wv_sb[:, :, kt, :],
                            start=False, stop=(kt == KT - 1), perf_mode=DR,
                        )
                    nc.vector.tensor_scalar(v_sb[:, st, 0:128], ps[:, 0:128],
                                            inv_ab, None, OP.mult)
                    nc.vector.tensor_scalar(v_sb[:, st, 129:257],
                                            ps[:, 128:256], inv_ab, None,
                                            OP.mult)

                def proj_q_groups(c):
                    return [lambda mt=mt: emit_proj_qk(
                        wq_sb, qT_sb, mt, c, 1.0 / (AQ * BX))
                        for mt in range(2)]

                def proj_kv_groups(c):
                    gs = [lambda mt=mt: emit_proj_qk(
                        wk_sb, kT_sb, mt, c, 1.0 / (AK * BX))
                        for mt in range(2)]
                    for st in range(4 * c, 4 * c + 4):
                        gs.append(lambda st=st: emit_proj_v(st, 1.0 / (AV * BX)))
                    return gs

                def proj_chunk_groups(c):
                    gs = []
                    qg = proj_q_groups(c)
                    kg = proj_kv_groups(c)
                    gs += [qg[0], kg[0], qg[1], kg[1]] + kg[2:]
                    return gs

                # ---------------- attention ----------------
                def pv_qtile(h, s, i, e_tiles, oq_s, ss_s):
                    qt = 4 * s + i
                    up = ps_u.tile([128, 258], f32, tag="u")
                    for kt in range(qt + 1):
                        c = i * 128
                        vh = v_sb[:, kt, 129 * h:129 * h + 129]
                        nc.tensor.matmul(
                            up[:, 0:129],
                            lhsT=e_tiles[kt][:, c:c + 128],
                            rhs=vh,
                            start=(kt == 0), stop=(kt == qt),
                        )
                        nc.tensor.matmul(
                            up[:, 129:258],
                            lhsT=e_tiles[kt][:, 512 + c:512 + c + 128],
                            rhs=vh,
                            start=False, stop=(kt == qt),
                            skip_group_check=True,
                        )
                    # normalized diff: oq = u0/s0 - lam*u1/s1 (per-partition)
                    inv0 = nrm.tile([128, 1], f32, tag="inv0")
                    nc.vector.reciprocal(inv0, up[:, 128:129])
                    inv1 = nrm.tile([128, 1], f32, tag="inv1")
                    nc.vector.reciprocal(inv1, up[:, 257:258])
                    t1 = nrm.tile([128, 128], f32, tag="t1")
                    nc.vector.tensor_scalar(t1, up[:, 129:257], inv1, lam,
                                            OP.mult, OP.mult)
                    oq = oq_s[:, i, :]
                    nc.vector.scalar_tensor_tensor(
                        oq, up[:, 0:128], inv0, t1, OP.mult, OP.subtract)
                    sq = nrm.tile([128, 128], f32, tag="sq")
                    nc.vector.scalar_tensor_tensor(
                        sq, oq, 1.0, oq, OP.bypass, OP.mult,
                        accum_out=ss_s[:, i:i + 1])

                def norm_tail(h, s, oq_s, ss_s, tpool=None, ttag="po",
                              j0=0, n=4):
                    """Newton rsqrt over n of the strip's q-tiles, then scale,
                    transpose and evacuate o^T."""
                    if tpool is None:
                        tpool = po
                    sl = slice(j0, j0 + n)
                    ms = nrm.tile([128, 4], f32, tag="ms")
                    il2 = 1.0 / ((1.0 - LAMBDA_INIT) ** 2)
                    nc.vector.tensor_scalar(ms[:, sl], ss_s[:, sl],
                                            il2 / 128.0, EPS * il2,
                                            OP.mult, OP.add)
                    y0 = nrm.tile([128, 4], u32, tag="y0")
                    nc.vector.tensor_tensor(y0[:, sl], ms.bitcast(u32)[:, sl],
                                            one_u[:, sl],
                                            OP.logical_shift_right)
                    nc.vector.tensor_tensor(y0[:, sl], magic_c[:, sl],
                                            y0[:, sl], OP.subtract)
                    yf = y0.bitcast(f32)
                    t2 = nrm.tile([128, 4], f32, tag="t2")
                    r_all = nrm.tile([128, 4], f32, tag="r_all")
                    nc.vector.tensor_mul(t2[:, sl], yf[:, sl], yf[:, sl])
                    nc.vector.tensor_mul(t2[:, sl], t2[:, sl], ms[:, sl])
                    nc.vector.tensor_scalar(t2[:, sl], t2[:, sl], -0.5, 1.5,
                                            OP.mult, OP.add)
                    nc.vector.tensor_mul(r_all[:, sl], yf[:, sl], t2[:, sl])
                    nc.vector.tensor_mul(t2[:, sl], r_all[:, sl],
                                         r_all[:, sl])
                    nc.vector.tensor_mul(t2[:, sl], t2[:, sl], ms[:, sl])
                    nc.vector.tensor_scalar(t2[:, sl], t2[:, sl], -0.5, 1.5,
                                            OP.mult, OP.add)
                    nc.vector.tensor_mul(r_all[:, sl], r_all[:, sl],
                                         t2[:, sl])
                    for i in range(j0, j0 + n):
                        qt = 4 * s + i
                        on = osb.tile([128, 128], f16, tag="on")
                        nc.vector.tensor_scalar(on, oq_s[:, i, :],
                                                r_all[:, i:i + 1], None,
                                                OP.mult)
                        pt = tpool.tile([128, 512], f16, tag=ttag)
                        nc.tensor.transpose(pt[:, 0:128], on, ident)
                        nc.vector.tensor_copy(
                            oT_sb[:, h, qt * 128:(qt + 1) * 128], pt[:, 0:128])

                def outproj_groups(s):
                    """8 filler groups (one per mt) computing strip s's
                    out-projection; interleaved into strip s+1."""
                    state = {}

                    def grp(mt):
                        if mt == 0:
                            ot = out_pool.tile([128, 8, 512], f16, tag="ot")
                            state["ot"] = ot
                        ot = state["ot"]
                        ps = po.tile([128, 512], f32, tag="po")
                        for kt in range(2):
                            nc.tensor.matmul(
                                ps[:],
                                lhsT=wo_sb[:, kt, mt * 128:(mt + 1) * 128],
                                rhs=oT_sb[:, kt, s * 512:(s + 1) * 512],
                                start=(kt == 0), stop=(kt == 1),
                            )
                        nc.vector.tensor_copy(ot[:, mt, :], ps[:])
                        if mt == 3:
                            nc.sync.dma_start(
                                outT_r[:, 0:4, s * 512:(s + 1) * 512],
                                ot[:, 0:4, :])
                        elif mt == 7:
                            nc.sync.dma_start(
                                outT_r[:, 4:8, s * 512:(s + 1) * 512],
                                ot[:, 4:8, :])

                    return [lambda mt=mt: grp(mt) for mt in range(8)]

                def outproj_last(s):
                    """Final strip: per-128-column sub-tile matmuls so 3/4 of
                    the PE work overlaps the final norm_tail; grouped evacs
                    alternate DVE/ACT; DMA per 2 mt."""
                    ot = out_pool.tile([128, 8, 512], f16, tag="ot")
                    for mt in range(8):
                        # borrow the idle scores/u psum slots: 6 groups in
                        # flight instead of 2
                        if mt % 3 == 0:
                            ps = po.tile([128, 512], f32, tag="po")
                        elif mt % 3 == 1:
                            ps = ps_s.tile([128, 512], f32, tag="sc")
                        else:
                            ps = ps_u.tile([128, 512], f32, tag="u")
                        for sub in range(4):
                            for kt in range(2):
                                nc.tensor.matmul(
                                    ps[:, sub * 128:(sub + 1) * 128],
                                    lhsT=wo_sb[:, kt, mt * 128:(mt + 1) * 128],
                                    rhs=oT_sb[:, kt,
                                              s * 512 + sub * 128:
                                              s * 512 + (sub + 1) * 128],
                                    start=(kt == 0), stop=(kt == 1),
                                    skip_group_check=(sub > 0),
                                )
                        if mt % 2 == 1:
                            nc.scalar.copy(ot[:, mt, :], ps[:])
                            q = nc.sync if mt % 4 == 1 else nc.gpsimd
                            q.dma_start(
                                outT_r[:, mt - 1:mt + 1,
                                       s * 512:(s + 1) * 512],
                                ot[:, mt - 1:mt + 1, :])
                        else:
                            nc.vector.tensor_copy(ot[:, mt, :], ps[:])

                pre_tiles = {}

                def early_sc_group(h, kt, strip=3, pool=None):
                    sl = strip * 512
                    pa = ps_s.tile([128, 1024], f32, tag="sc")
                    nc.tensor.matmul(
                        pa[:, 0:512],
                        lhsT=kT_sb[0:64, h, kt * 128:(kt + 1) * 128],
                        rhs=qT_sb[0:64, h, sl:sl + 512],
                        start=True, stop=True, tile_position=(0, 0),
                    )
                    nc.tensor.matmul(
                        pa[:, 512:1024],
                        lhsT=kT_sb[64:128, h, kt * 128:(kt + 1) * 128],
                        rhs=qT_sb[64:128, h, sl:sl + 512],
                        start=True, stop=True, tile_position=(64, 0),
                        skip_group_check=True,
                    )
                    if pool is None:
                        ee = pre_pool.tile([128, 1024], f16, tag="pe3")
                    else:
                        ee = pool.tile([128, 1024], f16, tag="e")
                    nc.scalar.activation(ee, pa, AF.Exp)
                    pre_tiles[(strip, h, kt)] = ee

                NPRE = 6  # strip-3 kt tiles precomputed per head

                # chunk 0 projections up front; the first two groups'
                # main chains run before any correction chain so the PE
                # isn't blocked in-order on the residual (v=1) DMAs
                ps_q0 = proj_qk_main(wq_sb, 0, 0)
                ps_k0 = proj_qk_main(wk_sb, 0, 0)
                proj_qk_corr(ps_q0, wq_sb, qT_sb, 0, 0, 1.0 / (AQ * BX))
                proj_qk_corr(ps_k0, wk_sb, kT_sb, 0, 0, 1.0 / (AK * BX))
                emit_proj_qk(wq_sb, qT_sb, 1, 0, 1.0 / (AQ * BX))
                emit_proj_qk(wk_sb, kT_sb, 1, 0, 1.0 / (AK * BX))
                for st in range(4):
                    emit_proj_v(st, 1.0 / (AV * BX))

                for s in range(NSTRIP):
                    pending = []
                    if s == 0:
                        pending += proj_chunk_groups(1)
                        pending += proj_q_groups(2) + proj_q_groups(3)
                        pending += [lambda h=h, kt=kt: early_sc_group(
                                        h, kt, strip=1, pool=epools[h])
                                    for h in range(2) for kt in range(4)]
                    elif s == 1:
                        early = [lambda h=h, kt=kt: early_sc_group(h, kt)
                                 for h in range(2) for kt in range(NPRE)]
                        early += [lambda h=h, kt=kt: early_sc_group(
                                      h, kt, strip=2, pool=epools[h])
                                  for h in range(2) for kt in range(2)]
                        rest = outproj_groups(0) + proj_kv_groups(2)
                        pending = [g for pair in zip(early, rest)
                                   for g in pair]
                        pending += early[len(rest):] + rest[len(early):]
                    elif s == 2:
                        pending += outproj_groups(1) + proj_kv_groups(3)
                    else:
                        pending += outproj_groups(2)
                    niter = 2 * 4 * (s + 1)
                    total0 = len(pending)
                    emitted = 0
                    it = 0
                    for h in range(2):
                        e_tiles = {}
                        oq_s = nrm_big.tile([128, 4, 128], f32, tag="oq_s")
                        ss_s = nrm_big.tile([128, 4], f32, tag="ss_s")
                        for kt in range(4 * (s + 1)):
                            npre_s = (NPRE if s == 3 else
                                      4 if s == 1 else
                                      2 if s == 2 else 0)
                            if kt < npre_s:
                                e_tiles[kt] = pre_tiles[(s, h, kt)]
                                it += 1
                                target = -(-total0 * it // niter)  # ceil
                                while pending and emitted < target:
                                    pending.pop(0)()
                                    emitted += 1
                                continue
                            col0 = max(0, (kt - 4 * s) * 128)
                            pa = ps_s.tile([128, 1024], f32, tag="sc")
                            nc.tensor.matmul(
                                pa[:, col0:512],
                                lhsT=kT_sb[0:64, h, kt * 128:(kt + 1) * 128],
                                rhs=qT_sb[0:64, h, s * 512 + col0:(s + 1) * 512],
                                start=True, stop=True, tile_position=(0, 0),
                            )
                            nc.tensor.matmul(
                                pa[:, 512 + col0:1024],
                                lhsT=kT_sb[64:128, h, kt * 128:(kt + 1) * 128],
                                rhs=qT_sb[64:128, h, s * 512 + col0:(s + 1) * 512],
                                start=True, stop=True, tile_position=(64, 0),
                                skip_group_check=True,
                            )
                            ee = epools[h].tile([128, 1024], f16, tag="e")
                            # exp both half-heads in one ACT op (strided AP
                            # skips the invalid leading columns of each half)
                            nc.scalar.activation(
                                ee.rearrange("p (b c) -> p b c", b=2)[:, :, col0:512],
                                pa.rearrange("p (b c) -> p b c", b=2)[:, :, col0:512],
                                AF.Exp)
                            if kt >= 4 * s:
                                c = col0
                                nc.gpsimd.tensor_mul(ee[:, c:c + 128],
                                                     ee[:, c:c + 128], maskt)
                                nc.gpsimd.tensor_mul(ee[:, 512 + c:512 + c + 128],
                                                     ee[:, 512 + c:512 + c + 128],
                                                     maskt)
                            e_tiles[kt] = ee
                            if kt >= 4 * s:
                                i = kt - 4 * s
                                pv_qtile(h, s, i, e_tiles, oq_s, ss_s)
                                if s == NSTRIP - 1 and h == 1 and i == 2:
                                    norm_tail(h, s, oq_s, ss_s, tpool=ps_s,
                                              ttag="sc", j0=0, n=3)
                            # interleave deferred work
                            it += 1
                            target = -(-total0 * it // niter)  # ceil
                            while pending and emitted < target:
                                pending.pop(0)()
                                emitted += 1
                        if s == NSTRIP - 1 and h == 1:
                            norm_tail(h, s, oq_s, ss_s, tpool=ps_s,
                                      ttag="sc", j0=3, n=1)
                        else:
                            norm_tail(h, s, oq_s, ss_s)

                outproj_last(NSTRIP - 1)

    nc.compile()
    return nc


def _prep_inputs(x, Wq, Wk, Wv, Wo):
    """Build the 8 per-core input maps (host-side shard + fp8 packing)."""
    import ml_dtypes
    F8 = ml_dtypes.float8_e4m3
    f16 = np.float16

    def q8(a):
        return a.astype(F8)

    scale = HD ** -0.5
    # x packs are per batch: [D, 2, S] with var 0 = x8, var 1 = r8
    xpacks = []
    for b in range(B):
        xT = np.ascontiguousarray(x[b].T) * BX     # (D, S) fp32
        x8 = q8(xT)
        r8 = q8(xT - x8.astype(np.float32))
        xp = np.empty((2, D, S), dtype=F8)
        xp[0] = x8
        xp[1] = r8
        xpacks.append(xp)

    def wpack(W, a, pre=1.0):
        # W slice already (rows 256, D); computes pack of (W.T * pre) * a
        WT = np.ascontiguousarray(W.T) * (pre * a)  # (D, 256)
        W8 = q8(WT)
        rW8 = q8(WT - W8.astype(np.float32))
        wp = np.empty((2, D, 256), dtype=F8)
        wp[0] = rW8
        wp[1] = W8
        return wp

    in_maps = []
    for d in range(N_CORES):
        b, p = divmod(d, 4)
        r0 = 256 * p
        in_maps.append({
            "xpack": xpacks[b],
            "wqp": wpack(Wq[r0:r0 + 256, :], AQ, pre=scale),
            "wkp": wpack(Wk[r0:r0 + 256, :], AK),
            "wvp": wpack(Wv[r0:r0 + 256, :], AV),
            "woT": np.ascontiguousarray(Wo[:, r0:r0 + 256].T).astype(f16),
        })
    return in_maps


_CACHED = {}


def _get_program(lam: float):
    # the program depends on inputs only through lam
    key = round(float(lam), 9)
    if key not in _CACHED:
        _CACHED[key] = _build_program(float(lam))
    return _CACHED[key]


def kernel(x, Wq, Wk, Wv, Wo, lq1, lk1, lq2, lk2):
    from concourse.bass_utils import run_bass_kernel_spmd

    x = np.asarray(x, dtype=np.float32)
    Wq = np.asarray(Wq, dtype=np.float32)
    Wk = np.asarray(Wk, dtype=np.float32)
    Wv = np.asarray(Wv, dtype=np.float32)
    Wo = np.asarray(Wo, dtype=np.float32)
    lq1 = np.asarray(lq1, dtype=np.float32)
    lk1 = np.asarray(lk1, dtype=np.float32)
    lq2 = np.asarray(lq2, dtype=np.float32)
    lk2 = np.asarray(lk2, dtype=np.float32)

    lam1 = np.exp(np.sum(lq1 * lk1, dtype=np.float32))
    lam2 = np.exp(np.sum(lq2 * lk2, dtype=np.float32))
    lam = float(lam1 - lam2 + LAMBDA_INIT)

    nc = _get_program(lam)
    in_maps = _prep_inputs(x, Wq, Wk, Wv, Wo)
    res = run_bass_kernel_spmd(nc, in_maps, core_ids=list(range(N_CORES)))

    out = np.empty((B, S, D), dtype=np.float32)
    for b in range(B):
        acc = res.results[4 * b]["outT"].astype(np.float32)
        for p in range(1, 4):
            acc += res.results[4 * b + p]["outT"].astype(np.float32)
        out[b] = acc.T
    return out
